# revision 18
# baseline (speedup 1.0000x reference)
"""Trainium2 Bass kernel for a pre-norm transformer block (RMSNorm + causal
RoPE attention + RMSNorm + SiLU FFN), distributed over 8 NeuronCores.

Sharding: phase 1 is head-parallel (2 of 16 heads per core, all tokens);
phase 2 is TWO AllToAlls (one per batch, ~0.5MB each) that redistribute
attention outputs from head-shards to token-shards and overlap with
compute; phase 3 (out-proj, residuals, FFN) is token-parallel and split
into two 256-token halves so half `a` runs while AllToAll `b` is in
flight.

Key deviations from the straightforward schedule:
- softmax normalisation is deferred past the collective: each shard ships
  the un-normalised numerator plus the denominator row (65 rows per head)
  and phase 3 divides after the redistribute, so phase 1 has no per-block
  reciprocals on its critical path.
- rmsnorm rstd = exp(-0.5*ln(var)) on the scalar engine: Ln and Exp share
  one activation-table set, so no Sqrt table swaps mid-kernel.
- the two heads' score tiles share one 2-bank PSUM tile, so exp and the
  causal mask run once per key-chunk instead of once per head.

RoPE is applied with a "swap projection" epilogue (see csr/snr), with head
dims host-permuted to [evens; odds] which leaves attention scores
invariant.
"""

import sys
import time
import numpy as np
import ml_dtypes
from contextlib import ExitStack

for _p in ("/opt/trn_rl_repo", "/root/.axon_site/_ro/trn_rl_repo"):
    if _p not in sys.path:
        sys.path.insert(0, _p)

import concourse.bass as bass
import concourse.tile as tile
from concourse import mybir

F32 = mybir.dt.float32
BF16 = mybir.dt.bfloat16
BF = ml_dtypes.bfloat16
AF = mybir.ActivationFunctionType
MUL = mybir.AluOpType.mult

B, S, D, H, DH = 2, 2048, 1024, 16, 64
FF = 2048
NCORES = 8
TLOC = 512                        # tokens per core in phase 3 (256 per batch)
THALF = 256
NQB = S // 512                    # 4 query blocks of 512 per batch
NDC = D // 128                    # 8 feature chunks
NFC = FF // 128                   # 16 ffn chunks
SCALE = 1.0 / float(np.sqrt(DH))
EPS = 1e-6
ROPE_BASE = 10000.0
SHROW = 130                       # rows per A2A shard: 2 heads x (64 num + 1 den)

_MAX_WAITS = 1


def _split_excess_waits(nc, max_waits=_MAX_WAITS):
    """walrus rejects >~2 sync-wait commands on one instruction; split the
    extras onto NoOps inserted just before, on the same engine."""
    counter = [0]

    def fresh_nop(engine, waits):
        counter[0] += 1
        nop = mybir.InstNoOp(name=f"I-waitsplit-{counter[0]}")
        nop.engine = engine
        nop.sync_info = mybir.SyncInfo(on_wait=list(waits), on_update=[])
        return nop

    for f in nc.m.functions:
        for bb in f.blocks:
            new_insts = []
            changed = False
            for inst in bb.instructions:
                si = inst.sync_info
                if si is not None and si.on_wait and len(si.on_wait) > max_waits:
                    waits = list(si.on_wait)
                    while len(waits) > max_waits:
                        chunk, waits = waits[:max_waits], waits[max_waits:]
                        new_insts.append(fresh_nop(inst.engine, chunk))
                    inst.sync_info = mybir.SyncInfo(
                        on_wait=waits, on_update=list(si.on_update or [])
                    )
                    changed = True
                new_insts.append(inst)
            if changed:
                bb.instructions[:] = new_insts


def _build_nc(debug=False):
    nc = bass.Bass("TRN2", target_bir_lowering=False, debug=False, num_devices=NCORES)

    xt_bf = nc.dram_tensor("xt_bf", [B, D, S], BF16, kind="ExternalInput")
    xt_loc = nc.dram_tensor("xt_loc", [D, TLOC], BF16, kind="ExternalInput")
    wqkv = nc.dram_tensor("wqkv", [128, 3, NDC, 128], BF16, kind="ExternalInput")
    wout = nc.dram_tensor("wout", [128, NDC, NDC, 128], BF16, kind="ExternalInput")
    wfc = nc.dram_tensor("wfc", [128, NDC, NFC, 128], BF16, kind="ExternalInput")
    wproj = nc.dram_tensor("wproj", [128, NFC, NDC, 128], BF16, kind="ExternalInput")
    cs_t = nc.dram_tensor("cs_t", [128, S], BF16, kind="ExternalInput")
    sn_t = nc.dram_tensor("sn_t", [128, S], BF16, kind="ExternalInput")
    wff = nc.dram_tensor("wff", [128, NDC], F32, kind="ExternalInput")
    ident_in = nc.dram_tensor("ident_in", [128, 128], BF16, kind="ExternalInput")
    sel16_in = nc.dram_tensor("sel16_in", [16, D], BF16, kind="ExternalInput")

    out_loc = nc.dram_tensor("out_loc", [D, TLOC], F32, kind="ExternalOutput")

    with tile.TileContext(nc) as tc, ExitStack() as top:
        # ---- single PSUM pool, tag-sized: 2x2(sc) + 2(num) + 2(misc) = 8 banks
        ps = top.enter_context(tc.tile_pool(name="ps", bufs=1, space="PSUM"))
        dram = top.enter_context(tc.tile_pool(name="dram", bufs=1, space="DRAM"))

        cc_in = [
            dram.tile([NCORES * SHROW, THALF], BF16, tag=f"cc_in{b}", name=f"cc_in{b}")
            for b in range(B)
        ]
        cc_out = [
            dram.tile([NCORES * SHROW, THALF], BF16, tag=f"cc_out{b}", name=f"cc_out{b}")
            for b in range(B)
        ]

        consts = top.enter_context(tc.tile_pool(name="consts", bufs=1))
        ones_col = consts.tile([128, 1], BF16, tag="ones_col")
        nc.vector.memset(ones_col[:], 1.0)
        ones_row = consts.tile([1, 128], F32, tag="ones_row")
        nc.vector.memset(ones_row[:], 1.0)
        eps_sb = consts.tile([1, 1], F32, tag="eps_sb")
        nc.vector.memset(eps_sb[:], EPS)
        prime = consts.tile([1, 1], F32, tag="prime")
        nc.scalar.activation(out=prime[:], in_=eps_sb[:], func=AF.Ln)
        ident = consts.tile([128, 128], BF16, tag="ident")
        nc.gpsimd.dma_start(ident[:], ident_in[:])
        sel16 = consts.tile([16, NDC, 128], BF16, tag="sel16")
        nc.gpsimd.dma_start(sel16[:], sel16_in[:].rearrange("p (c m) -> p c m", m=128))

        wpre = top.enter_context(tc.tile_pool(name="wpre", bufs=1))
        wout_sb = wpre.tile([128, NDC, NDC, 128], BF16, tag="wout")
        wfc_sb = wpre.tile([128, NDC, NFC, 128], BF16, tag="wfc")
        wproj_sb = wpre.tile([128, NFC, NDC, 128], BF16, tag="wproj")

        # ================= phase 1: head-parallel attention =================
        persist = top.enter_context(tc.tile_pool(name="persist", bufs=1))
        cs_sb = persist.tile([128, S], BF16, tag="cs")
        sn_sb = persist.tile([128, S], BF16, tag="sn")
        wqkv_sb = persist.tile([128, 3, NDC, 128], BF16, tag="wqkv")
        nc.sync.dma_start(wqkv_sb[:], wqkv[:])

        bp = top.enter_context(tc.tile_pool(name="bp", bufs=1))
        xq = top.enter_context(tc.tile_pool(name="xq", bufs=3))
        rbc = top.enter_context(tc.tile_pool(name="rbc", bufs=3))
        tmp = top.enter_context(tc.tile_pool(name="tmp", bufs=2))
        sqp = top.enter_context(tc.tile_pool(name="sqp", bufs=1))
        tmp2 = top.enter_context(tc.tile_pool(name="tmp2", bufs=1))
        att_sb_pool = top.enter_context(tc.tile_pool(name="attsb", bufs=2))
        ptp = top.enter_context(tc.tile_pool(name="ptp", bufs=2))

        na_tiles = {}

        def norm_a(b, qb):
            """load x for query block (b, qb) and square it on the DVE (keeps
            the gpsimd queue free for the causal masks)."""
            qs = slice(qb * 512, (qb + 1) * 512)
            x_q = xq.tile([128, NDC, 512], BF16, tag="x_q", name="x_q")
            nc.sync.dma_start(
                x_q[:],
                xt_bf[b, :, qs].rearrange("(c p) f -> p c f", p=128),
            )
            sq = sqp.tile([128, NDC, 512], BF16, tag="sq", name="sq")
            nc.vector.tensor_mul(sq[:], x_q[:], x_q[:])
            na_tiles[(b, qb)] = (x_q, sq)

        def norm_b(b, qb):
            """variance + rstd for a block whose x^2 is already in flight.
            rstd = exp(-0.5*ln(var/D + eps)): Ln+Exp share one table set."""
            x_q, sq = na_tiles.pop((b, qb))
            var_ps = ps.tile([1, 512], F32, tag="misc", name="var")
            for dc in range(NDC):
                nc.tensor.matmul(
                    var_ps[:], ones_col[:], sq[:, dc, :],
                    start=(dc == 0), stop=(dc == NDC - 1),
                )
            lnv = tmp.tile([1, 512], F32, tag="st", name="lnv")
            nc.scalar.activation(
                out=lnv[:], in_=var_ps[:], func=AF.Ln,
                scale=1.0 / D, bias=eps_sb[:],
            )
            rstd = tmp.tile([1, 512], F32, tag="st", name="rstd")
            nc.scalar.activation(out=rstd[:], in_=lnv[:], func=AF.Exp, scale=-0.5)
            rstd_ps = ps.tile([128, 512], F32, tag="misc", name="rstd_ps")
            nc.tensor.matmul(rstd_ps[:], ones_row[:], rstd[:], start=True, stop=True)
            rstd_bc = rbc.tile([128, 512], BF16, tag="rstd_bc", name="rstd_bc")
            nc.vector.tensor_copy(rstd_bc[:], rstd_ps[:])
            h_tiles[(b, qb)] = (x_q, rstd_bc)

        steps = [(0, 0), (0, 1), (1, 0), (1, 1), (0, 2), (0, 3), (1, 2), (1, 3)]
        batch_tiles = {}
        h_tiles = {}

        def ensure_batch_tiles(b):
            if b not in batch_tiles:
                q_rot = bp.tile([128, S], BF16, tag="q_rot", name="q_rot", bufs=2)
                k_rot = bp.tile([128, S], BF16, tag="k_rot", name="k_rot", bufs=2)
                v_aug = bp.tile([128, S // 128, 2, 65], BF16, tag="v_aug", name="v_aug", bufs=2)
                nc.vector.memset(v_aug[:, :, :, 64:65], 1.0)
                batch_tiles[b] = (q_rot, k_rot, v_aug)
            return batch_tiles[b]

        SWAP16 = list(range(16, 32)) + list(range(16))

        def qkv_chunks(b, qb):
            """qkv projections + fused rmsnorm scaling + rope + v transpose,
            as a list of emission thunks so the work interleaves with the
            previous step's attention inner loop (fills exp-gated PE gaps)."""
            qs = slice(qb * 512, (qb + 1) * 512)
            q_rot, k_rot, v_aug = ensure_batch_tiles(b)
            st = {}

            def proj_half(rc, half, key):
                if half == 0:
                    st[key] = ps.tile([128, 512], F32, tag="proj", name=f"mm_{key}")
                for dc in range(half * 4, half * 4 + 4):
                    nc.tensor.matmul(
                        st[key][:], wqkv_sb[:, rc, dc, :], st["x_q"][:, dc, :],
                        start=(dc == 0), stop=(dc == NDC - 1),
                    )

            def c_start():
                st["x_q"], st["rstd_bc"] = h_tiles.pop((b, qb))
                rstd_bc = st["rstd_bc"]
                csr = tmp2.tile([128, 512], BF16, tag="csr", name="csr")
                nc.vector.tensor_mul(csr[:], cs_sb[:, qs], rstd_bc[:])
                snr = tmp2.tile([128, 512], BF16, tag="snr", name="snr")
                nc.vector.tensor_mul(snr[:], sn_sb[:, qs], rstd_bc[:])
                st["csr"], st["snr"] = csr, snr
                proj_half(0, 0, "q")

            def rope(key, dst):
                p_main = st[key]
                t1 = tmp2.tile([128, 512], BF16, tag="t1", name="t1", bufs=2)
                nc.vector.tensor_mul(t1[:], p_main[:], st["csr"][:])
                qsh = tmp2.tile([128, 512], F32, tag="qsh", name="qsh")
                nc.vector.stream_shuffle(qsh[:], p_main[:], SWAP16)
                t2 = tmp2.tile([128, 512], BF16, tag="t2", name="t2")
                nc.vector.tensor_mul(t2[:], qsh[:], st["snr"][:])
                nc.vector.tensor_add(dst[:, qs], t1[:], t2[:])

            def c_vepi():
                p_v = st["v"]
                v_f = tmp2.tile([128, 512], BF16, tag="t1", name="v_f", bufs=2)
                nc.vector.tensor_mul(v_f[:], p_v[:], st["rstd_bc"][:])
                for j in range(4):
                    kc = qb * 4 + j
                    tr_ps = ps.tile([128, 128], BF16, tag="misc", name="tr_ps")
                    nc.tensor.transpose(
                        tr_ps[:], v_f[:, j * 128:(j + 1) * 128], ident[:]
                    )
                    nc.vector.tensor_copy(
                        v_aug[:, kc, :, 0:64],
                        tr_ps[:].rearrange("p (h d) -> p h d", h=2),
                    )

            return [
                c_start,
                lambda: (proj_half(0, 1, "q"), rope("q", q_rot)),
                lambda: proj_half(1, 0, "k"),
                lambda: (proj_half(1, 1, "k"), rope("k", k_rot)),
                lambda: proj_half(2, 0, "v"),
                lambda: (proj_half(2, 1, "v"), c_vepi()),
            ]

        def attention_block(si, b, qb, feeds):
            qs = slice(qb * 512, (qb + 1) * 512)
            q_rot, k_rot, v_aug = batch_tiles[b]
            nkc = 4 * (qb + 1)
            num_h = [
                ps.tile([65, 512], F32, tag=f"num{hh}", name=f"num{hh}")
                for hh in range(2)
            ]
            for kc in range(nkc):
                ks = slice(kc * 128, (kc + 1) * 128)
                dlt = kc - qb * 4
                sc_ps = ps.tile([128, 2, 512], F32, tag="sc", name="sc_ps", bufs=2)
                for hh in range(2):
                    hs = slice(hh * 64, (hh + 1) * 64)
                    nc.tensor.matmul(
                        sc_ps[:, hh, :], k_rot[hs, ks], q_rot[hs, qs],
                        start=True, stop=True,
                        tile_position=(hh * 64, 0),
                    )
                pT = ptp.tile([128, 2, 512], BF16, tag="pT", name="pT")
                nc.scalar.activation(
                    out=pT[:], in_=sc_ps[:], func=AF.Exp, scale=SCALE
                )
                if dlt >= 0:
                    # causal: zero probs where query < key (same mask both heads)
                    nc.gpsimd.affine_select(
                        out=pT[:], in_=pT[:],
                        compare_op=mybir.AluOpType.is_ge,
                        fill=0.0, base=-dlt * 128,
                        channel_multiplier=-1,
                        pattern=[[0, 2], [1, 512]],
                    )
                # interleave next-step qkv/norm emission into the exp-gated gap
                n_feed = -(-len(feeds) // (nkc - kc))  # ceil
                for _ in range(n_feed):
                    feeds.pop(0)()
                for hh in range(2):
                    nc.tensor.matmul(
                        num_h[hh][:], v_aug[:, kc, hh, :], pT[:, hh, :],
                        start=(kc == 0), stop=(kc == nkc - 1),
                    )
            # ship un-normalised numerator + denominator row; divide in phase 3
            bi = 2 * b + (qb % 2)
            ccx = cc_in[0] if qb < 2 else cc_in[1]
            for hh in range(2):
                att65 = att_sb_pool.tile([65, 512], BF16, tag="att65", name="att65")
                nc.vector.tensor_copy(att65[:], num_h[hh][:])
                for half in range(2):
                    dst = 2 * bi + half
                    nc.sync.dma_start(
                        ccx[dst * SHROW + hh * 65: dst * SHROW + (hh + 1) * 65, :],
                        att65[:, half * THALF:(half + 1) * THALF],
                    )

        def emit_a2a(b):
            nc.gpsimd.collective_compute(
                "AllToAll",
                mybir.AluOpType.bypass,
                replica_groups=[list(range(NCORES))],
                ins=[cc_in[b][:]],
                outs=[cc_out[b][:]],
            )

        # ============ phase 3: token-parallel out-proj + FFN (one half) =====
        p3 = top.enter_context(tc.tile_pool(name="p3", bufs=1))
        p3t = top.enter_context(tc.tile_pool(name="p3t", bufs=2))
        wff_sb = p3.tile([128, NDC], F32, tag="wff")
        nc.scalar.dma_start(wff_sb[:], wff[:])

        p3_tiles = {}

        def phase3_load(b):
            toff = b * THALF
            cco = cc_out[b]
            at_all = p3.tile([128, NDC, THALF], BF16, tag="at_all")
            xl = p3.tile([128, NDC, THALF], BF16, tag="xl")
            for hh in range(2):
                nc.sync.dma_start(
                    at_all[:].rearrange("(h r) c f -> h r c f", h=2)[hh],
                    cco[:].rearrange("(c h r) f -> h r c f", c=NCORES, h=2)[
                        hh, 0:64
                    ],
                )
            nc.sync.dma_start(
                xl[:],
                xt_loc[:, toff:toff + THALF].rearrange("(c p) f -> p c f", p=128),
            )
            # softmax denominators: 16 rows in one strided DMA. den_bf lives in
            # a phase-1 pool slot so the scheduler cannot hoist this DMA (and
            # the DVE chain behind it) ahead of the remaining phase-1 work.
            den_bf = p3.tile([16, THALF], BF16, tag="den_bf")
            nc.gpsimd.dma_start(
                den_bf[:],
                cco[:].rearrange("(c h r) f -> r (c h) f", c=NCORES, h=2)[64],
            )
            p3_tiles[b] = (at_all, xl, den_bf)

        def phase3_compute(b):
            toff = b * THALF
            at_all, xl, den_bf = p3_tiles.pop(b)
            den32 = att_sb_pool.tile([16, THALF], F32, tag="att65", name="den32")
            nc.vector.tensor_copy(den32[:], den_bf[:])
            rcp16 = att_sb_pool.tile([16, THALF], F32, tag="att65", name="rcp16")
            nc.vector.reciprocal(rcp16[:], den32[:])
            rcp16b = att_sb_pool.tile([16, THALF], BF16, tag="att65", name="rcp16b")
            nc.vector.tensor_copy(rcp16b[:], rcp16[:])
            # broadcast per-head reciprocal to 128 rows and normalise in place
            for dc in range(NDC):
                rcp_ps = ps.tile([128, THALF], F32, tag="num1", name="rcp_ps")
                nc.tensor.matmul(
                    rcp_ps[:], sel16[:, dc, :], rcp16b[:], start=True, stop=True
                )
                nc.vector.tensor_mul(at_all[:, dc, :], at_all[:, dc, :], rcp_ps[:])

            x1_all = p3.tile([128, NDC, THALF], BF16, tag="x1")
            h2_all = p3.tile([128, NDC, THALF], BF16, tag="h2", name="h2_all")
            hid_all = p3.tile([128, NFC, THALF], BF16, tag="hid")

            # out-proj + residual, with norm2 stats interleaved per chunk
            var2 = ps.tile([1, THALF], F32, tag="num0", name="var2")
            for ec in range(NDC):
                op_ps = ps.tile([128, 2, 512], F32, tag="sc", name="op_ps", bufs=2)
                for dc in range(NDC):
                    nc.tensor.matmul(
                        op_ps[:, 0, 0:THALF], wout_sb[:, dc, ec, :], at_all[:, dc, :],
                        start=(dc == 0), stop=(dc == NDC - 1),
                    )
                nc.vector.tensor_add(x1_all[:, ec, :], op_ps[:, 0, 0:THALF], xl[:, ec, :])
                sq2 = p3t.tile([128, THALF], BF16, tag="sq2", name="sq2")
                nc.vector.tensor_mul(sq2[:], x1_all[:, ec, :], x1_all[:, ec, :])
                nc.tensor.matmul(
                    var2[:], ones_col[:], sq2[:],
                    start=(ec == 0), stop=(ec == NDC - 1),
                )

            # rmsnorm 2 via ln/exp (same act table set as phase-1 exp)
            lnv2 = p3t.tile([1, THALF], F32, tag="st2", name="lnv2")
            nc.scalar.activation(
                out=lnv2[:], in_=var2[:], func=AF.Ln, scale=1.0 / D, bias=eps_sb[:]
            )
            rstd2 = p3t.tile([1, THALF], F32, tag="st2", name="rstd2")
            nc.scalar.activation(out=rstd2[:], in_=lnv2[:], func=AF.Exp, scale=-0.5)
            rstd2_ps = ps.tile([128, THALF], F32, tag="misc", name="rstd2_ps")
            nc.tensor.matmul(rstd2_ps[:], ones_row[:], rstd2[:], start=True, stop=True)
            rstd2_bc = p3t.tile([128, THALF], BF16, tag="rstd2_bc")
            nc.vector.tensor_copy(rstd2_bc[:], rstd2_ps[:])
            for ec in range(NDC):
                nc.vector.scalar_tensor_tensor(
                    out=h2_all[:, ec, :],
                    in0=x1_all[:, ec, :],
                    scalar=wff_sb[:, ec:ec + 1],
                    in1=rstd2_bc[:],
                    op0=MUL, op1=MUL,
                )

            # fc + silu
            for fi in range(NFC):
                fc_ps = ps.tile([128, 2, 512], F32, tag="sc", name="fc_ps", bufs=2)
                for ec in range(NDC):
                    nc.tensor.matmul(
                        fc_ps[:, 0, 0:THALF], wfc_sb[:, ec, fi, :], h2_all[:, ec, :],
                        start=(ec == 0), stop=(ec == NDC - 1),
                    )
                nc.scalar.activation(
                    out=hid_all[:, fi, :], in_=fc_ps[:, 0, 0:THALF], func=AF.Silu
                )

            # proj + residual + store
            for ec in range(NDC):
                pr_ps = ps.tile([128, 2, 512], F32, tag="sc", name="pr_ps", bufs=2)
                for fi in range(NFC):
                    nc.tensor.matmul(
                        pr_ps[:, 0, 0:THALF], wproj_sb[:, fi, ec, :], hid_all[:, fi, :],
                        start=(fi == 0), stop=(fi == NFC - 1),
                    )
                y = p3t.tile([128, THALF], F32, tag="y")
                nc.vector.tensor_add(y[:], pr_ps[:, 0, 0:THALF], x1_all[:, ec, :])
                nc.sync.dma_start(
                    out_loc[ec * 128:(ec + 1) * 128, toff:toff + THALF], y[:]
                )

        # ================= emission schedule ===============================
        # prologue: x+x^2 for steps 0/1, stats for step 0, full qkv for step 0;
        # then attention(si) feeds [stats(si+1), x+x^2(si+2), qkv(si+1)] into
        # its exp-gated PE gaps chunk by chunk.
        norm_a(*steps[0])
        nc.scalar.dma_start(cs_sb[:], cs_t[:])
        nc.scalar.dma_start(sn_sb[:], sn_t[:])
        norm_a(*steps[1])
        norm_b(*steps[0])
        for chunk in qkv_chunks(*steps[0]):
            chunk()

        for si, (b, qb) in enumerate(steps):
            feeds = []
            if si + 1 < len(steps):
                sb, sqb = steps[si + 1]
                feeds.append(lambda sb=sb, sqb=sqb: norm_b(sb, sqb))
            if si + 2 < len(steps):
                sb, sqb = steps[si + 2]
                feeds.append(lambda sb=sb, sqb=sqb: norm_a(sb, sqb))
            if si + 1 < len(steps):
                feeds.extend(qkv_chunks(*steps[si + 1]))
            # big phase-3 weights: staggered so they don't starve the
            # startup-critical x/cs/sn loads of HBM bandwidth
            if si == 1:
                nc.gpsimd.dma_start(wout_sb[:], wout[:])
            if si == 2:
                nc.gpsimd.dma_start(wfc_sb[:], wfc[:])
            if si == 3:
                nc.gpsimd.dma_start(wproj_sb[:], wproj[:])
            attention_block(si, b, qb, feeds)
            if si == 3:
                emit_a2a(0)          # qb{0,1} redistribute, overlaps qb{2,3} attn

        phase3_load(0)
        emit_a2a(1)
        phase3_load(1)
        phase3_compute(0)            # overlaps the second AllToAll
        phase3_compute(1)

    _split_excess_waits(nc)
    return nc


# ---------------------------------------------------------------------------
# host-side prep


def _rope_tables():
    half = DH // 2
    inv_freq = 1.0 / (ROPE_BASE ** (2.0 * np.arange(half, dtype=np.float32) / DH))
    angles = np.arange(S, dtype=np.float32)[:, None] * inv_freq[None, :]  # (S, 32)
    cosT = np.cos(angles).T.astype(np.float32)  # (32, S) rows=freq
    sinT = np.sin(angles).T.astype(np.float32)
    # per head 64 rows = [e0..15, o0..15 | e16..31, o16..31]
    cs64 = np.concatenate([cosT[0:16], cosT[0:16], cosT[16:32], cosT[16:32]], axis=0)
    sn64 = np.concatenate([-sinT[0:16], sinT[0:16], -sinT[16:32], sinT[16:32]], axis=0)
    return (
        np.ascontiguousarray(np.tile(cs64, (2, 1))).astype(BF),
        np.ascontiguousarray(np.tile(sn64, (2, 1))).astype(BF),
    )  # (128, S)


def _prep_core_inputs(x, w_in_norm, w_ff_norm, w_qkv, w_out, w_fc, w_proj):
    x = np.asarray(x, dtype=np.float32)
    w_qkv = np.asarray(w_qkv, dtype=np.float32)
    w_out = np.asarray(w_out, dtype=np.float32)
    w_fc = np.asarray(w_fc, dtype=np.float32)
    w_proj = np.asarray(w_proj, dtype=np.float32)
    w_in_norm = np.asarray(w_in_norm, dtype=np.float32)
    w_ff_norm = np.asarray(w_ff_norm, dtype=np.float32)

    w_q, w_k, w_v = w_qkv[0:D], w_qkv[D:2 * D], w_qkv[2 * D:3 * D]

    xt = np.ascontiguousarray(x.transpose(0, 2, 1))        # (B, D, S)
    xt_bf = xt.astype(BF)

    cs_t, sn_t = _rope_tables()
    ident = np.eye(128, dtype=np.float32).astype(BF)
    # head selector for the phase-3 denominator broadcast: sel16[k, m] = [m//64 == k]
    sel16 = (np.arange(D)[None, :] // DH == np.arange(16)[:, None]).astype(BF)

    # SBUF layout [p, dc, ec, m]: element = w.T[dc*128+p, ec*128+m]
    wout_h = np.ascontiguousarray(
        w_out.T.reshape(NDC, 128, NDC, 128).transpose(1, 0, 2, 3)
    ).astype(BF)
    wfc_h = np.ascontiguousarray(
        w_fc.T.reshape(NDC, 128, NFC, 128).transpose(1, 0, 2, 3)
    ).astype(BF)
    wproj_h = np.ascontiguousarray(
        w_proj.T.reshape(NFC, 128, NDC, 128).transpose(1, 0, 2, 3)
    ).astype(BF)
    wff_h = np.ascontiguousarray(w_ff_norm.reshape(NDC, 128).T)

    ev = np.arange(0, DH, 2)
    od = np.arange(1, DH, 2)

    per_core = []
    for c in range(NCORES):
        hs = [2 * c, 2 * c + 1]

        def perm_rows(wm):
            # per head: [e0..15, o0..15, e16..31, o16..31]
            rows = []
            for h in hs:
                base = h * DH
                rows.append(wm[base + ev[0:16]])
                rows.append(wm[base + od[0:16]])
                rows.append(wm[base + ev[16:32]])
                rows.append(wm[base + od[16:32]])
            return np.concatenate(rows, axis=0)     # (128, D)

        def nat_rows(wm):
            return np.concatenate([wm[h * DH:(h + 1) * DH] for h in hs], axis=0)

        w_loc = np.stack([perm_rows(w_q), perm_rows(w_k), nat_rows(w_v)])  # (3, 128, D)
        w_loc = w_loc * w_in_norm[None, None, :]  # fold rmsnorm weight into qkv
        # SBUF layout [p, rc, dc, m]: element = w_loc[rc].T[dc*128+p, m]
        wqkv_h = np.ascontiguousarray(
            w_loc.transpose(0, 2, 1).reshape(3, NDC, 128, 128).transpose(2, 0, 1, 3)
        ).astype(BF)

        # phase-3 ownership: half x of core c = 256 tokens from query block
        # qb=(c//2)%2+2x of batch c//4, quarter (c%2)
        b_c, qb_c, q0 = c // 4, (c // 2) % 2, (c % 2) * THALF
        xt_loc = np.concatenate(
            [
                xt[b_c, :, (qb_c + 2 * x) * 512 + q0: (qb_c + 2 * x) * 512 + q0 + THALF]
                for x in range(2)
            ],
            axis=1,
        ).astype(BF)

        per_core.append({
            "xt_bf": xt_bf,
            "xt_loc": np.ascontiguousarray(xt_loc),
            "wqkv": wqkv_h,
            "wout": wout_h,
            "wfc": wfc_h,
            "wproj": wproj_h,
            "cs_t": cs_t,
            "sn_t": sn_t,
            "wff": wff_h,
            "ident_in": ident,
            "sel16_in": sel16,
        })
    return per_core


def _assemble(outs):
    full = np.empty((B, S, D), dtype=np.float32)
    for c in range(NCORES):
        b_c, qb_c, q0 = c // 4, (c // 2) % 2, (c % 2) * THALF
        for x in range(2):
            t0 = (qb_c + 2 * x) * 512 + q0
            full[b_c, t0:t0 + THALF, :] = (
                outs[c]["out_loc"][:, x * THALF:(x + 1) * THALF].T
            )
    return full


_CACHE = {}


def _get_runner(debug=False):
    """Build the Bass module + a cached jitted shard_map executor, so repeated
    kernel() calls do not recompile."""
    key = ("runner", debug)
    if key in _CACHE:
        return _CACHE[key]

    nc = _build_nc(debug=debug)

    import jax
    from jax.sharding import Mesh, PartitionSpec
    from jax.experimental.shard_map import shard_map
    from concourse import bass2jax

    bass2jax.install_neuronx_cc_hook()

    in_names, out_names, out_avals, zero_outs = [], [], [], []
    for alloc in nc.m.functions[0].allocations:
        if not isinstance(alloc, mybir.MemoryLocationSet):
            continue
        name = alloc.memorylocations[0].name
        if alloc.kind == "ExternalInput":
            in_names.append(name)
        elif alloc.kind == "ExternalOutput":
            out_names.append(name)
            shape = tuple(alloc.tensor_shape)
            dtype = mybir.dt.np(alloc.dtype)
            out_avals.append(jax.core.ShapedArray(shape, dtype))
            zero_outs.append(np.zeros(shape, dtype))
    partition_name = nc.partition_id_tensor.name if nc.partition_id_tensor else None
    if partition_name is not None and partition_name in in_names:
        in_names.remove(partition_name)
    n_params = len(in_names)
    n_outs = len(out_avals)
    all_in_names = in_names + out_names
    if partition_name is not None:
        all_in_names = all_in_names + [partition_name]

    def _body(*args):
        operands = list(args)
        if partition_name is not None:
            operands.append(bass2jax.partition_id_tensor())
        outs = bass2jax._bass_exec_p.bind(
            *operands,
            out_avals=tuple(out_avals),
            in_names=tuple(all_in_names),
            out_names=tuple(out_names),
            lowering_input_output_aliases=(),
            sim_require_finite=True,
            sim_require_nnan=True,
            nc=nc,
        )
        return tuple(outs)

    devices = jax.devices()[:NCORES]
    mesh = Mesh(np.asarray(devices), ("core",))
    donate = tuple(range(n_params, n_params + n_outs))
    sharded = jax.jit(
        shard_map(
            _body,
            mesh=mesh,
            in_specs=(PartitionSpec("core"),) * (n_params + n_outs),
            out_specs=(PartitionSpec("core"),) * n_outs,
            check_rep=False,
        ),
        donate_argnums=donate,
        keep_unused=True,
    )

    def runner(in_maps):
        concat_in = [
            np.concatenate([np.asarray(m[name]) for m in in_maps], axis=0)
            for name in in_names
        ]
        concat_zeros = [
            np.zeros((NCORES * z.shape[0], *z.shape[1:]), z.dtype) for z in zero_outs
        ]
        out_arrs = sharded(*concat_in, *concat_zeros)
        return [
            {
                name: np.asarray(out_arrs[i]).reshape(NCORES, *out_avals[i].shape)[c]
                for i, name in enumerate(out_names)
            }
            for c in range(NCORES)
        ]

    _CACHE[key] = runner
    _CACHE[("runner_meta", debug)] = (sharded, in_names, out_avals, zero_outs, mesh)
    return runner


def kernel(**inputs) -> np.ndarray:
    per_core = _prep_core_inputs(**inputs)
    runner = _get_runner(debug=False)
    outs = runner(per_core)
    return _assemble(outs)


def time_kernel(iters=5, **inputs):
    """Wall-clock the jitted sharded execution with device-resident inputs.
    Returns best-of-iters nanoseconds (includes dispatch overhead, so it is
    an upper bound on HW kernel time)."""
    import jax

    per_core = _prep_core_inputs(**inputs)
    runner = _get_runner(debug=False)
    meta = _CACHE[("runner_meta", False)]
    sharded, in_names, out_avals, zero_outs, mesh = meta

    from jax.sharding import NamedSharding, PartitionSpec

    sh = NamedSharding(mesh, PartitionSpec("core"))
    concat_in = [
        np.concatenate([np.asarray(m[name]) for m in per_core], axis=0)
        for name in in_names
    ]
    dev_in = [jax.device_put(a, sh) for a in concat_in]

    def fresh_zeros():
        return [
            jax.device_put(
                np.zeros((NCORES * z.shape[0], *z.shape[1:]), z.dtype), sh
            )
            for z in zero_outs
        ]

    # warm
    out = sharded(*dev_in, *fresh_zeros())
    jax.block_until_ready(out)
    best = None
    for _ in range(iters):
        zs = fresh_zeros()
        jax.block_until_ready(zs)
        t0 = time.perf_counter_ns()
        out = sharded(*dev_in, *zs)
        jax.block_until_ready(out)
        t1 = time.perf_counter_ns()
        best = t1 - t0 if best is None else min(best, t1 - t0)
    return best


if __name__ == "__main__":
    rng = np.random.default_rng(0)
    ins = {
        "x": rng.standard_normal((B, S, D), dtype=np.float32),
        "w_in_norm": np.ones(D, np.float32),
        "w_ff_norm": np.ones(D, np.float32),
        "w_qkv": (rng.standard_normal((3 * D, D), dtype=np.float32) / 32),
        "w_out": (rng.standard_normal((D, D), dtype=np.float32) / 32),
        "w_fc": (rng.standard_normal((FF, D), dtype=np.float32) / 32),
        "w_proj": (rng.standard_normal((D, FF), dtype=np.float32) / np.sqrt(FF).astype(np.float32)),
    }
    out = kernel(**ins)
    print("out", out.shape, out.dtype, float(np.abs(out).mean()))


# revision 19
# speedup vs baseline: 1.0141x; 1.0141x over previous
"""Trainium2 Bass kernel for a pre-norm transformer block (RMSNorm + causal
RoPE attention + RMSNorm + SiLU FFN), distributed over 8 NeuronCores.

Sharding: phase 1 is head-parallel (2 of 16 heads per core, all tokens);
phase 2 is TWO AllToAlls (one per batch, ~0.5MB each) that redistribute
attention outputs from head-shards to token-shards and overlap with
compute; phase 3 (out-proj, residuals, FFN) is token-parallel and split
into two 256-token halves so half `a` runs while AllToAll `b` is in
flight.

Key deviations from the straightforward schedule:
- softmax normalisation is deferred past the collective: each shard ships
  the un-normalised numerator plus the denominator row (65 rows per head)
  and phase 3 divides after the redistribute, so phase 1 has no per-block
  reciprocals on its critical path.
- rmsnorm rstd = exp(-0.5*ln(var)) on the scalar engine: Ln and Exp share
  one activation-table set, so no Sqrt table swaps mid-kernel.
- the two heads' score tiles share one 2-bank PSUM tile, so exp and the
  causal mask run once per key-chunk instead of once per head.

RoPE is applied with a "swap projection" epilogue (see csr/snr), with head
dims host-permuted to [evens; odds] which leaves attention scores
invariant.
"""

import sys
import time
import numpy as np
import ml_dtypes
from contextlib import ExitStack

for _p in ("/opt/trn_rl_repo", "/root/.axon_site/_ro/trn_rl_repo"):
    if _p not in sys.path:
        sys.path.insert(0, _p)

import concourse.bass as bass
import concourse.tile as tile
from concourse import mybir

F32 = mybir.dt.float32
BF16 = mybir.dt.bfloat16
BF = ml_dtypes.bfloat16
AF = mybir.ActivationFunctionType
MUL = mybir.AluOpType.mult

B, S, D, H, DH = 2, 2048, 1024, 16, 64
FF = 2048
NCORES = 8
TLOC = 512                        # tokens per core in phase 3 (256 per batch)
THALF = 256
NQB = S // 512                    # 4 query blocks of 512 per batch
NDC = D // 128                    # 8 feature chunks
NFC = FF // 128                   # 16 ffn chunks
SCALE = 1.0 / float(np.sqrt(DH))
EPS = 1e-6
ROPE_BASE = 10000.0
SHROW = 130                       # rows per A2A shard: 2 heads x (64 num + 1 den)

_MAX_WAITS = 1


def _split_excess_waits(nc, max_waits=_MAX_WAITS):
    """walrus rejects >~2 sync-wait commands on one instruction; split the
    extras onto NoOps inserted just before, on the same engine."""
    counter = [0]

    def fresh_nop(engine, waits):
        counter[0] += 1
        nop = mybir.InstNoOp(name=f"I-waitsplit-{counter[0]}")
        nop.engine = engine
        nop.sync_info = mybir.SyncInfo(on_wait=list(waits), on_update=[])
        return nop

    for f in nc.m.functions:
        for bb in f.blocks:
            new_insts = []
            changed = False
            for inst in bb.instructions:
                si = inst.sync_info
                if si is not None and si.on_wait and len(si.on_wait) > max_waits:
                    waits = list(si.on_wait)
                    while len(waits) > max_waits:
                        chunk, waits = waits[:max_waits], waits[max_waits:]
                        new_insts.append(fresh_nop(inst.engine, chunk))
                    inst.sync_info = mybir.SyncInfo(
                        on_wait=waits, on_update=list(si.on_update or [])
                    )
                    changed = True
                new_insts.append(inst)
            if changed:
                bb.instructions[:] = new_insts


def _build_nc(debug=False):
    nc = bass.Bass("TRN2", target_bir_lowering=False, debug=False, num_devices=NCORES)

    xt_bf = nc.dram_tensor("xt_bf", [B * NQB, 128, NDC, 512], BF16, kind="ExternalInput")
    xt_loc = nc.dram_tensor("xt_loc", [2, 128, NDC, THALF], BF16, kind="ExternalInput")
    wqkv = nc.dram_tensor("wqkv", [128, 3, NDC, 128], BF16, kind="ExternalInput")
    wout = nc.dram_tensor("wout", [128, NDC, NDC, 128], BF16, kind="ExternalInput")
    wfc = nc.dram_tensor("wfc", [128, NDC, NFC, 128], BF16, kind="ExternalInput")
    wproj = nc.dram_tensor("wproj", [128, NFC, NDC, 128], BF16, kind="ExternalInput")
    cs_t = nc.dram_tensor("cs_t", [128, S], BF16, kind="ExternalInput")
    sn_t = nc.dram_tensor("sn_t", [128, S], BF16, kind="ExternalInput")
    wff = nc.dram_tensor("wff", [128, NDC], F32, kind="ExternalInput")
    ident_in = nc.dram_tensor("ident_in", [128, 128], BF16, kind="ExternalInput")
    sel16_in = nc.dram_tensor("sel16_in", [16, D], BF16, kind="ExternalInput")

    out_loc = nc.dram_tensor("out_loc", [2, 128, NDC, THALF], F32, kind="ExternalOutput")

    with tile.TileContext(nc) as tc, ExitStack() as top:
        # ---- single PSUM pool, tag-sized: 2x2(sc) + 2(num) + 2(misc) = 8 banks
        ps = top.enter_context(tc.tile_pool(name="ps", bufs=1, space="PSUM"))
        dram = top.enter_context(tc.tile_pool(name="dram", bufs=1, space="DRAM"))

        cc_in = [
            dram.tile([NCORES * SHROW, THALF], BF16, tag=f"cc_in{b}", name=f"cc_in{b}")
            for b in range(B)
        ]
        cc_out = [
            dram.tile([NCORES * SHROW, THALF], BF16, tag=f"cc_out{b}", name=f"cc_out{b}")
            for b in range(B)
        ]

        consts = top.enter_context(tc.tile_pool(name="consts", bufs=1))
        ones_col = consts.tile([128, 1], BF16, tag="ones_col")
        nc.vector.memset(ones_col[:], 1.0)
        ones_row = consts.tile([1, 128], F32, tag="ones_row")
        nc.vector.memset(ones_row[:], 1.0)
        eps_sb = consts.tile([1, 1], F32, tag="eps_sb")
        nc.vector.memset(eps_sb[:], EPS)
        prime = consts.tile([1, 1], F32, tag="prime")
        nc.scalar.activation(out=prime[:], in_=eps_sb[:], func=AF.Ln)
        ident = consts.tile([128, 128], BF16, tag="ident")
        nc.gpsimd.dma_start(ident[:], ident_in[:])
        sel16 = consts.tile([16, NDC, 128], BF16, tag="sel16")
        nc.gpsimd.dma_start(sel16[:], sel16_in[:].rearrange("p (c m) -> p c m", m=128))

        wpre = top.enter_context(tc.tile_pool(name="wpre", bufs=1))
        wout_sb = wpre.tile([128, NDC, NDC, 128], BF16, tag="wout")
        wfc_sb = wpre.tile([128, NDC, NFC, 128], BF16, tag="wfc")
        wproj_sb = wpre.tile([128, NFC, NDC, 128], BF16, tag="wproj")

        # ================= phase 1: head-parallel attention =================
        persist = top.enter_context(tc.tile_pool(name="persist", bufs=1))
        cs_sb = persist.tile([128, S], BF16, tag="cs")
        sn_sb = persist.tile([128, S], BF16, tag="sn")
        wqkv_sb = persist.tile([128, 3, NDC, 128], BF16, tag="wqkv")
        nc.sync.dma_start(wqkv_sb[:], wqkv[:])

        bp = top.enter_context(tc.tile_pool(name="bp", bufs=1))
        xq = top.enter_context(tc.tile_pool(name="xq", bufs=3))
        rbc = top.enter_context(tc.tile_pool(name="rbc", bufs=3))
        tmp = top.enter_context(tc.tile_pool(name="tmp", bufs=2))
        sqp = top.enter_context(tc.tile_pool(name="sqp", bufs=1))
        tmp2 = top.enter_context(tc.tile_pool(name="tmp2", bufs=1))
        att_sb_pool = top.enter_context(tc.tile_pool(name="attsb", bufs=2))
        ptp = top.enter_context(tc.tile_pool(name="ptp", bufs=2))

        na_tiles = {}

        def norm_a(b, qb):
            """load x for query block (b, qb) and square it on the DVE (keeps
            the gpsimd queue free for the causal masks)."""
            qs = slice(qb * 512, (qb + 1) * 512)
            x_q = xq.tile([128, NDC, 512], BF16, tag="x_q", name="x_q")
            nc.sync.dma_start(x_q[:], xt_bf[b * NQB + qb])
            sq = sqp.tile([128, NDC, 512], BF16, tag="sq", name="sq")
            nc.vector.tensor_mul(sq[:], x_q[:], x_q[:])
            na_tiles[(b, qb)] = (x_q, sq)

        def norm_b(b, qb):
            """variance + rstd for a block whose x^2 is already in flight.
            rstd = exp(-0.5*ln(var/D + eps)): Ln+Exp share one table set."""
            x_q, sq = na_tiles.pop((b, qb))
            var_ps = ps.tile([1, 512], F32, tag="misc", name="var")
            for dc in range(NDC):
                nc.tensor.matmul(
                    var_ps[:], ones_col[:], sq[:, dc, :],
                    start=(dc == 0), stop=(dc == NDC - 1),
                )
            lnv = tmp.tile([1, 512], F32, tag="st", name="lnv")
            nc.scalar.activation(
                out=lnv[:], in_=var_ps[:], func=AF.Ln,
                scale=1.0 / D, bias=eps_sb[:],
            )
            rstd = tmp.tile([1, 512], F32, tag="st", name="rstd")
            nc.scalar.activation(out=rstd[:], in_=lnv[:], func=AF.Exp, scale=-0.5)
            rstd_ps = ps.tile([128, 512], F32, tag="misc", name="rstd_ps")
            nc.tensor.matmul(rstd_ps[:], ones_row[:], rstd[:], start=True, stop=True)
            rstd_bc = rbc.tile([128, 512], BF16, tag="rstd_bc", name="rstd_bc")
            nc.vector.tensor_copy(rstd_bc[:], rstd_ps[:])
            h_tiles[(b, qb)] = (x_q, rstd_bc)

        steps = [(0, 0), (0, 1), (1, 0), (1, 1), (0, 2), (0, 3), (1, 2), (1, 3)]
        batch_tiles = {}
        h_tiles = {}

        def ensure_batch_tiles(b):
            if b not in batch_tiles:
                q_rot = bp.tile([128, S], BF16, tag="q_rot", name="q_rot", bufs=2)
                k_rot = bp.tile([128, S], BF16, tag="k_rot", name="k_rot", bufs=2)
                v_aug = bp.tile([128, S // 128, 2, 65], BF16, tag="v_aug", name="v_aug", bufs=2)
                nc.vector.memset(v_aug[:, :, :, 64:65], 1.0)
                batch_tiles[b] = (q_rot, k_rot, v_aug)
            return batch_tiles[b]

        SWAP16 = list(range(16, 32)) + list(range(16))

        def qkv_chunks(b, qb):
            """qkv projections + fused rmsnorm scaling + rope + v transpose,
            as a list of emission thunks so the work interleaves with the
            previous step's attention inner loop (fills exp-gated PE gaps)."""
            qs = slice(qb * 512, (qb + 1) * 512)
            q_rot, k_rot, v_aug = ensure_batch_tiles(b)
            st = {}

            def proj_half(rc, half, key):
                if half == 0:
                    st[key] = ps.tile([128, 512], F32, tag="proj", name=f"mm_{key}")
                for dc in range(half * 4, half * 4 + 4):
                    nc.tensor.matmul(
                        st[key][:], wqkv_sb[:, rc, dc, :], st["x_q"][:, dc, :],
                        start=(dc == 0), stop=(dc == NDC - 1),
                    )

            def c_start():
                st["x_q"], st["rstd_bc"] = h_tiles.pop((b, qb))
                rstd_bc = st["rstd_bc"]
                csr = tmp2.tile([128, 512], BF16, tag="csr", name="csr")
                nc.vector.tensor_mul(csr[:], cs_sb[:, qs], rstd_bc[:])
                snr = tmp2.tile([128, 512], BF16, tag="snr", name="snr")
                nc.vector.tensor_mul(snr[:], sn_sb[:, qs], rstd_bc[:])
                st["csr"], st["snr"] = csr, snr
                proj_half(0, 0, "q")

            def rope(key, dst):
                p_main = st[key]
                t1 = tmp2.tile([128, 512], BF16, tag="t1", name="t1", bufs=2)
                nc.vector.tensor_mul(t1[:], p_main[:], st["csr"][:])
                qsh = tmp2.tile([128, 512], F32, tag="qsh", name="qsh")
                nc.vector.stream_shuffle(qsh[:], p_main[:], SWAP16)
                t2 = tmp2.tile([128, 512], BF16, tag="t2", name="t2")
                nc.vector.tensor_mul(t2[:], qsh[:], st["snr"][:])
                nc.vector.tensor_add(dst[:, qs], t1[:], t2[:])

            def c_vepi():
                p_v = st["v"]
                v_f = tmp2.tile([128, 512], BF16, tag="t1", name="v_f", bufs=2)
                nc.vector.tensor_mul(v_f[:], p_v[:], st["rstd_bc"][:])
                for j in range(4):
                    kc = qb * 4 + j
                    tr_ps = ps.tile([128, 128], BF16, tag="misc", name="tr_ps")
                    nc.tensor.transpose(
                        tr_ps[:], v_f[:, j * 128:(j + 1) * 128], ident[:]
                    )
                    nc.vector.tensor_copy(
                        v_aug[:, kc, :, 0:64],
                        tr_ps[:].rearrange("p (h d) -> p h d", h=2),
                    )

            return [
                c_start,
                lambda: (proj_half(0, 1, "q"), rope("q", q_rot)),
                lambda: proj_half(1, 0, "k"),
                lambda: (proj_half(1, 1, "k"), rope("k", k_rot)),
                lambda: proj_half(2, 0, "v"),
                lambda: (proj_half(2, 1, "v"), c_vepi()),
            ]

        def attention_block(si, b, qb, feeds):
            qs = slice(qb * 512, (qb + 1) * 512)
            q_rot, k_rot, v_aug = batch_tiles[b]
            nkc = 4 * (qb + 1)
            num_h = [
                ps.tile([65, 512], F32, tag=f"num{hh}", name=f"num{hh}")
                for hh in range(2)
            ]
            for kc in range(nkc):
                ks = slice(kc * 128, (kc + 1) * 128)
                dlt = kc - qb * 4
                sc_ps = ps.tile([128, 2, 512], F32, tag="sc", name="sc_ps", bufs=2)
                for hh in range(2):
                    hs = slice(hh * 64, (hh + 1) * 64)
                    nc.tensor.matmul(
                        sc_ps[:, hh, :], k_rot[hs, ks], q_rot[hs, qs],
                        start=True, stop=True,
                        tile_position=(hh * 64, 0),
                    )
                pT = ptp.tile([128, 2, 512], BF16, tag="pT", name="pT")
                nc.scalar.activation(
                    out=pT[:], in_=sc_ps[:], func=AF.Exp, scale=SCALE
                )
                if dlt >= 0:
                    # causal: zero probs where query < key (same mask both heads)
                    nc.gpsimd.affine_select(
                        out=pT[:], in_=pT[:],
                        compare_op=mybir.AluOpType.is_ge,
                        fill=0.0, base=-dlt * 128,
                        channel_multiplier=-1,
                        pattern=[[0, 2], [1, 512]],
                    )
                # interleave next-step qkv/norm emission into the exp-gated gap
                n_feed = -(-len(feeds) // (nkc - kc))  # ceil
                for _ in range(n_feed):
                    feeds.pop(0)()
                for hh in range(2):
                    nc.tensor.matmul(
                        num_h[hh][:], v_aug[:, kc, hh, :], pT[:, hh, :],
                        start=(kc == 0), stop=(kc == nkc - 1),
                    )
            # ship un-normalised numerator + denominator row; divide in phase 3
            bi = 2 * b + (qb % 2)
            ccx = cc_in[0] if qb < 2 else cc_in[1]
            for hh in range(2):
                att65 = att_sb_pool.tile([65, 512], BF16, tag="att65", name="att65")
                nc.vector.tensor_copy(att65[:], num_h[hh][:])
                for half in range(2):
                    dst = 2 * bi + half
                    nc.sync.dma_start(
                        ccx[dst * SHROW + hh * 65: dst * SHROW + (hh + 1) * 65, :],
                        att65[:, half * THALF:(half + 1) * THALF],
                    )

        def emit_a2a(b):
            nc.gpsimd.collective_compute(
                "AllToAll",
                mybir.AluOpType.bypass,
                replica_groups=[list(range(NCORES))],
                ins=[cc_in[b][:]],
                outs=[cc_out[b][:]],
            )

        # ============ phase 3: token-parallel out-proj + FFN (one half) =====
        p3 = top.enter_context(tc.tile_pool(name="p3", bufs=1))
        p3t = top.enter_context(tc.tile_pool(name="p3t", bufs=2))
        wff_sb = p3.tile([128, NDC], F32, tag="wff")
        nc.scalar.dma_start(wff_sb[:], wff[:])

        p3_tiles = {}

        def phase3_load(b):
            cco = cc_out[b]
            at_all = p3.tile([128, NDC, THALF], BF16, tag="at_all")
            xl = p3.tile([128, NDC, THALF], BF16, tag="xl")
            for hh in range(2):
                nc.sync.dma_start(
                    at_all[:].rearrange("(h r) c f -> h r c f", h=2)[hh],
                    cco[:].rearrange("(c h r) f -> h r c f", c=NCORES, h=2)[
                        hh, 0:64
                    ],
                )
            nc.sync.dma_start(xl[:], xt_loc[b])
            # softmax denominators: 16 rows in one strided DMA. den_bf lives in
            # a phase-1 pool slot so the scheduler cannot hoist this DMA (and
            # the DVE chain behind it) ahead of the remaining phase-1 work.
            den_bf = p3.tile([16, THALF], BF16, tag="den_bf")
            nc.sync.dma_start(
                den_bf[:],
                cco[:].rearrange("(c h r) f -> r (c h) f", c=NCORES, h=2)[64],
            )
            p3_tiles[b] = (at_all, xl, den_bf)

        def phase3_compute(b):
            at_all, xl, den_bf = p3_tiles.pop(b)
            den32 = p3.tile([16, THALF], F32, tag="den32")
            nc.vector.tensor_copy(den32[:], den_bf[:])
            rcp16 = p3.tile([16, THALF], F32, tag="rcp16")
            nc.vector.reciprocal(rcp16[:], den32[:])
            rcp16b = att_sb_pool.tile([16, THALF], BF16, tag="att65", name="rcp16b")
            nc.vector.tensor_copy(rcp16b[:], rcp16[:])
            # broadcast per-head reciprocal to 128 rows and normalise in place
            for dc in range(NDC):
                rcp_ps = ps.tile([128, THALF], F32, tag="num1", name="rcp_ps")
                nc.tensor.matmul(
                    rcp_ps[:], sel16[:, dc, :], rcp16b[:], start=True, stop=True
                )
                nc.vector.tensor_mul(at_all[:, dc, :], at_all[:, dc, :], rcp_ps[:])

            x1_all = p3.tile([128, NDC, THALF], BF16, tag="x1")
            h2_all = p3.tile([128, NDC, THALF], BF16, tag="h2", name="h2_all")
            hid_all = p3.tile([128, NFC, THALF], BF16, tag="hid")

            # out-proj + residual, with norm2 stats interleaved per chunk
            var2 = ps.tile([1, THALF], F32, tag="num0", name="var2")
            for ec in range(NDC):
                op_ps = ps.tile([128, 2, 512], F32, tag="sc", name="op_ps", bufs=2)
                for dc in range(NDC):
                    nc.tensor.matmul(
                        op_ps[:, 0, 0:THALF], wout_sb[:, dc, ec, :], at_all[:, dc, :],
                        start=(dc == 0), stop=(dc == NDC - 1),
                    )
                nc.vector.tensor_add(x1_all[:, ec, :], op_ps[:, 0, 0:THALF], xl[:, ec, :])
                sq2 = p3t.tile([128, THALF], BF16, tag="sq2", name="sq2")
                nc.vector.tensor_mul(sq2[:], x1_all[:, ec, :], x1_all[:, ec, :])
                nc.tensor.matmul(
                    var2[:], ones_col[:], sq2[:],
                    start=(ec == 0), stop=(ec == NDC - 1),
                )

            # rmsnorm 2 via ln/exp (same act table set as phase-1 exp)
            lnv2 = p3t.tile([1, THALF], F32, tag="st2", name="lnv2")
            nc.scalar.activation(
                out=lnv2[:], in_=var2[:], func=AF.Ln, scale=1.0 / D, bias=eps_sb[:]
            )
            rstd2 = p3t.tile([1, THALF], F32, tag="st2", name="rstd2")
            nc.scalar.activation(out=rstd2[:], in_=lnv2[:], func=AF.Exp, scale=-0.5)
            rstd2_ps = ps.tile([128, THALF], F32, tag="misc", name="rstd2_ps")
            nc.tensor.matmul(rstd2_ps[:], ones_row[:], rstd2[:], start=True, stop=True)
            rstd2_bc = p3t.tile([128, THALF], BF16, tag="rstd2_bc")
            nc.vector.tensor_copy(rstd2_bc[:], rstd2_ps[:])
            for ec in range(NDC):
                nc.vector.scalar_tensor_tensor(
                    out=h2_all[:, ec, :],
                    in0=x1_all[:, ec, :],
                    scalar=wff_sb[:, ec:ec + 1],
                    in1=rstd2_bc[:],
                    op0=MUL, op1=MUL,
                )

            # fc + silu
            for fi in range(NFC):
                fc_ps = ps.tile([128, 2, 512], F32, tag="sc", name="fc_ps", bufs=2)
                for ec in range(NDC):
                    nc.tensor.matmul(
                        fc_ps[:, 0, 0:THALF], wfc_sb[:, ec, fi, :], h2_all[:, ec, :],
                        start=(ec == 0), stop=(ec == NDC - 1),
                    )
                nc.scalar.activation(
                    out=hid_all[:, fi, :], in_=fc_ps[:, 0, 0:THALF], func=AF.Silu
                )

            # proj + residual + store
            for ec in range(NDC):
                pr_ps = ps.tile([128, 2, 512], F32, tag="sc", name="pr_ps", bufs=2)
                for fi in range(NFC):
                    nc.tensor.matmul(
                        pr_ps[:, 0, 0:THALF], wproj_sb[:, fi, ec, :], hid_all[:, fi, :],
                        start=(fi == 0), stop=(fi == NFC - 1),
                    )
                y = p3t.tile([128, THALF], F32, tag="y")
                nc.vector.tensor_add(y[:], pr_ps[:, 0, 0:THALF], x1_all[:, ec, :])
                nc.sync.dma_start(out_loc[b, :, ec, :], y[:])

        # ================= emission schedule ===============================
        # prologue: x+x^2 for steps 0/1, stats for step 0, full qkv for step 0;
        # then attention(si) feeds [stats(si+1), x+x^2(si+2), qkv(si+1)] into
        # its exp-gated PE gaps chunk by chunk.
        norm_a(*steps[0])
        nc.scalar.dma_start(cs_sb[:], cs_t[:])
        nc.scalar.dma_start(sn_sb[:], sn_t[:])
        norm_a(*steps[1])
        norm_b(*steps[0])
        for chunk in qkv_chunks(*steps[0]):
            chunk()

        for si, (b, qb) in enumerate(steps):
            feeds = []
            if si + 1 < len(steps):
                sb, sqb = steps[si + 1]
                feeds.append(lambda sb=sb, sqb=sqb: norm_b(sb, sqb))
            if si + 2 < len(steps):
                sb, sqb = steps[si + 2]
                feeds.append(lambda sb=sb, sqb=sqb: norm_a(sb, sqb))
            if si + 1 < len(steps):
                feeds.extend(qkv_chunks(*steps[si + 1]))
            # big phase-3 weights: staggered so they don't starve the
            # startup-critical x/cs/sn loads of HBM bandwidth
            if si == 1:
                nc.gpsimd.dma_start(wout_sb[:], wout[:])
            if si == 2:
                nc.gpsimd.dma_start(wfc_sb[:], wfc[:])
            if si == 3:
                nc.gpsimd.dma_start(wproj_sb[:], wproj[:])
            attention_block(si, b, qb, feeds)
            if si == 3:
                emit_a2a(0)          # qb{0,1} redistribute, overlaps qb{2,3} attn

        with tc.tile_wait_until(5):
            phase3_load(0)
        emit_a2a(1)
        with tc.tile_wait_until(5):
            phase3_load(1)
            phase3_compute(0)        # overlaps the second AllToAll
            phase3_compute(1)

    _split_excess_waits(nc)
    return nc


# ---------------------------------------------------------------------------
# host-side prep


def _rope_tables():
    half = DH // 2
    inv_freq = 1.0 / (ROPE_BASE ** (2.0 * np.arange(half, dtype=np.float32) / DH))
    angles = np.arange(S, dtype=np.float32)[:, None] * inv_freq[None, :]  # (S, 32)
    cosT = np.cos(angles).T.astype(np.float32)  # (32, S) rows=freq
    sinT = np.sin(angles).T.astype(np.float32)
    # per head 64 rows = [e0..15, o0..15 | e16..31, o16..31]
    cs64 = np.concatenate([cosT[0:16], cosT[0:16], cosT[16:32], cosT[16:32]], axis=0)
    sn64 = np.concatenate([-sinT[0:16], sinT[0:16], -sinT[16:32], sinT[16:32]], axis=0)
    return (
        np.ascontiguousarray(np.tile(cs64, (2, 1))).astype(BF),
        np.ascontiguousarray(np.tile(sn64, (2, 1))).astype(BF),
    )  # (128, S)


def _prep_core_inputs(x, w_in_norm, w_ff_norm, w_qkv, w_out, w_fc, w_proj):
    x = np.asarray(x, dtype=np.float32)
    w_qkv = np.asarray(w_qkv, dtype=np.float32)
    w_out = np.asarray(w_out, dtype=np.float32)
    w_fc = np.asarray(w_fc, dtype=np.float32)
    w_proj = np.asarray(w_proj, dtype=np.float32)
    w_in_norm = np.asarray(w_in_norm, dtype=np.float32)
    w_ff_norm = np.asarray(w_ff_norm, dtype=np.float32)

    w_q, w_k, w_v = w_qkv[0:D], w_qkv[D:2 * D], w_qkv[2 * D:3 * D]

    xt = np.ascontiguousarray(x.transpose(0, 2, 1))        # (B, D, S)
    # block/partition-major: xt_pre[b*4+qb, p, c, f] = xt[b, c*128+p, qb*512+f],
    # so each 1MB x_q block DMA is contiguous per partition (8KB lines)
    xt_pre = np.ascontiguousarray(
        xt.reshape(B, NDC, 128, NQB, 512).transpose(0, 3, 2, 1, 4)
    ).reshape(B * NQB, 128, NDC, 512).astype(BF)

    cs_t, sn_t = _rope_tables()
    ident = np.eye(128, dtype=np.float32).astype(BF)
    # head selector for the phase-3 denominator broadcast: sel16[k, m] = [m//64 == k]
    sel16 = (np.arange(D)[None, :] // DH == np.arange(16)[:, None]).astype(BF)

    # SBUF layout [p, dc, ec, m]: element = w.T[dc*128+p, ec*128+m]
    wout_h = np.ascontiguousarray(
        w_out.T.reshape(NDC, 128, NDC, 128).transpose(1, 0, 2, 3)
    ).astype(BF)
    wfc_h = np.ascontiguousarray(
        w_fc.T.reshape(NDC, 128, NFC, 128).transpose(1, 0, 2, 3)
    ).astype(BF)
    wproj_h = np.ascontiguousarray(
        w_proj.T.reshape(NFC, 128, NDC, 128).transpose(1, 0, 2, 3)
    ).astype(BF)
    wff_h = np.ascontiguousarray(w_ff_norm.reshape(NDC, 128).T)

    ev = np.arange(0, DH, 2)
    od = np.arange(1, DH, 2)

    per_core = []
    for c in range(NCORES):
        hs = [2 * c, 2 * c + 1]

        def perm_rows(wm):
            # per head: [e0..15, o0..15, e16..31, o16..31]
            rows = []
            for h in hs:
                base = h * DH
                rows.append(wm[base + ev[0:16]])
                rows.append(wm[base + od[0:16]])
                rows.append(wm[base + ev[16:32]])
                rows.append(wm[base + od[16:32]])
            return np.concatenate(rows, axis=0)     # (128, D)

        def nat_rows(wm):
            return np.concatenate([wm[h * DH:(h + 1) * DH] for h in hs], axis=0)

        w_loc = np.stack([perm_rows(w_q), perm_rows(w_k), nat_rows(w_v)])  # (3, 128, D)
        w_loc = w_loc * w_in_norm[None, None, :]  # fold rmsnorm weight into qkv
        # SBUF layout [p, rc, dc, m]: element = w_loc[rc].T[dc*128+p, m]
        wqkv_h = np.ascontiguousarray(
            w_loc.transpose(0, 2, 1).reshape(3, NDC, 128, 128).transpose(2, 0, 1, 3)
        ).astype(BF)

        # phase-3 ownership: half x of core c = 256 tokens from query block
        # qb=(c//2)%2+2x of batch c//4, quarter (c%2); [x, p, c, f] layout
        b_c, qb_c, q0 = c // 4, (c // 2) % 2, (c % 2) * THALF
        xt_loc = np.stack(
            [
                xt[b_c, :, (qb_c + 2 * x) * 512 + q0: (qb_c + 2 * x) * 512 + q0 + THALF]
                .reshape(NDC, 128, THALF).transpose(1, 0, 2)
                for x in range(2)
            ]
        ).astype(BF)

        per_core.append({
            "xt_bf": xt_pre,
            "xt_loc": np.ascontiguousarray(xt_loc),
            "wqkv": wqkv_h,
            "wout": wout_h,
            "wfc": wfc_h,
            "wproj": wproj_h,
            "cs_t": cs_t,
            "sn_t": sn_t,
            "wff": wff_h,
            "ident_in": ident,
            "sel16_in": sel16,
        })
    return per_core


def _assemble(outs):
    full = np.empty((B, S, D), dtype=np.float32)
    for c in range(NCORES):
        b_c, qb_c, q0 = c // 4, (c // 2) % 2, (c % 2) * THALF
        for x in range(2):
            t0 = (qb_c + 2 * x) * 512 + q0
            # out_loc[x, p, cc, f] -> features cc*128+p
            full[b_c, t0:t0 + THALF, :] = (
                outs[c]["out_loc"][x].transpose(1, 0, 2).reshape(D, THALF).T
            )
    return full


_CACHE = {}


def _get_runner(debug=False):
    """Build the Bass module + a cached jitted shard_map executor, so repeated
    kernel() calls do not recompile."""
    key = ("runner", debug)
    if key in _CACHE:
        return _CACHE[key]

    nc = _build_nc(debug=debug)

    import jax
    from jax.sharding import Mesh, PartitionSpec
    from jax.experimental.shard_map import shard_map
    from concourse import bass2jax

    bass2jax.install_neuronx_cc_hook()

    in_names, out_names, out_avals, zero_outs = [], [], [], []
    for alloc in nc.m.functions[0].allocations:
        if not isinstance(alloc, mybir.MemoryLocationSet):
            continue
        name = alloc.memorylocations[0].name
        if alloc.kind == "ExternalInput":
            in_names.append(name)
        elif alloc.kind == "ExternalOutput":
            out_names.append(name)
            shape = tuple(alloc.tensor_shape)
            dtype = mybir.dt.np(alloc.dtype)
            out_avals.append(jax.core.ShapedArray(shape, dtype))
            zero_outs.append(np.zeros(shape, dtype))
    partition_name = nc.partition_id_tensor.name if nc.partition_id_tensor else None
    if partition_name is not None and partition_name in in_names:
        in_names.remove(partition_name)
    n_params = len(in_names)
    n_outs = len(out_avals)
    all_in_names = in_names + out_names
    if partition_name is not None:
        all_in_names = all_in_names + [partition_name]

    def _body(*args):
        operands = list(args)
        if partition_name is not None:
            operands.append(bass2jax.partition_id_tensor())
        outs = bass2jax._bass_exec_p.bind(
            *operands,
            out_avals=tuple(out_avals),
            in_names=tuple(all_in_names),
            out_names=tuple(out_names),
            lowering_input_output_aliases=(),
            sim_require_finite=True,
            sim_require_nnan=True,
            nc=nc,
        )
        return tuple(outs)

    devices = jax.devices()[:NCORES]
    mesh = Mesh(np.asarray(devices), ("core",))
    donate = tuple(range(n_params, n_params + n_outs))
    sharded = jax.jit(
        shard_map(
            _body,
            mesh=mesh,
            in_specs=(PartitionSpec("core"),) * (n_params + n_outs),
            out_specs=(PartitionSpec("core"),) * n_outs,
            check_rep=False,
        ),
        donate_argnums=donate,
        keep_unused=True,
    )

    def runner(in_maps):
        concat_in = [
            np.concatenate([np.asarray(m[name]) for m in in_maps], axis=0)
            for name in in_names
        ]
        concat_zeros = [
            np.zeros((NCORES * z.shape[0], *z.shape[1:]), z.dtype) for z in zero_outs
        ]
        out_arrs = sharded(*concat_in, *concat_zeros)
        return [
            {
                name: np.asarray(out_arrs[i]).reshape(NCORES, *out_avals[i].shape)[c]
                for i, name in enumerate(out_names)
            }
            for c in range(NCORES)
        ]

    _CACHE[key] = runner
    _CACHE[("runner_meta", debug)] = (sharded, in_names, out_avals, zero_outs, mesh)
    return runner


def kernel(**inputs) -> np.ndarray:
    per_core = _prep_core_inputs(**inputs)
    runner = _get_runner(debug=False)
    outs = runner(per_core)
    return _assemble(outs)


def time_kernel(iters=5, **inputs):
    """Wall-clock the jitted sharded execution with device-resident inputs.
    Returns best-of-iters nanoseconds (includes dispatch overhead, so it is
    an upper bound on HW kernel time)."""
    import jax

    per_core = _prep_core_inputs(**inputs)
    runner = _get_runner(debug=False)
    meta = _CACHE[("runner_meta", False)]
    sharded, in_names, out_avals, zero_outs, mesh = meta

    from jax.sharding import NamedSharding, PartitionSpec

    sh = NamedSharding(mesh, PartitionSpec("core"))
    concat_in = [
        np.concatenate([np.asarray(m[name]) for m in per_core], axis=0)
        for name in in_names
    ]
    dev_in = [jax.device_put(a, sh) for a in concat_in]

    def fresh_zeros():
        return [
            jax.device_put(
                np.zeros((NCORES * z.shape[0], *z.shape[1:]), z.dtype), sh
            )
            for z in zero_outs
        ]

    # warm
    out = sharded(*dev_in, *fresh_zeros())
    jax.block_until_ready(out)
    best = None
    for _ in range(iters):
        zs = fresh_zeros()
        jax.block_until_ready(zs)
        t0 = time.perf_counter_ns()
        out = sharded(*dev_in, *zs)
        jax.block_until_ready(out)
        t1 = time.perf_counter_ns()
        best = t1 - t0 if best is None else min(best, t1 - t0)
    return best


if __name__ == "__main__":
    rng = np.random.default_rng(0)
    ins = {
        "x": rng.standard_normal((B, S, D), dtype=np.float32),
        "w_in_norm": np.ones(D, np.float32),
        "w_ff_norm": np.ones(D, np.float32),
        "w_qkv": (rng.standard_normal((3 * D, D), dtype=np.float32) / 32),
        "w_out": (rng.standard_normal((D, D), dtype=np.float32) / 32),
        "w_fc": (rng.standard_normal((FF, D), dtype=np.float32) / 32),
        "w_proj": (rng.standard_normal((D, FF), dtype=np.float32) / np.sqrt(FF).astype(np.float32)),
    }
    out = kernel(**ins)
    print("out", out.shape, out.dtype, float(np.abs(out).mean()))


# revision 20
# speedup vs baseline: 1.0584x; 1.0436x over previous
"""Trainium2 Bass kernel for a pre-norm transformer block (RMSNorm + causal
RoPE attention + RMSNorm + SiLU FFN), distributed over 8 NeuronCores.

Sharding: phase 1 is head-parallel (2 of 16 heads per core, all tokens);
phase 2 is TWO AllToAlls (one per batch, ~0.5MB each) that redistribute
attention outputs from head-shards to token-shards and overlap with
compute; phase 3 (out-proj, residuals, FFN) is token-parallel and split
into two 256-token halves so half `a` runs while AllToAll `b` is in
flight.

Key deviations from the straightforward schedule:
- softmax normalisation is deferred past the collective: each shard ships
  the un-normalised numerator plus the denominator row (65 rows per head)
  and phase 3 divides after the redistribute, so phase 1 has no per-block
  reciprocals on its critical path.
- rmsnorm rstd = exp(-0.5*ln(var)) on the scalar engine: Ln and Exp share
  one activation-table set, so no Sqrt table swaps mid-kernel.
- the two heads' score tiles share one 2-bank PSUM tile, so exp and the
  causal mask run once per key-chunk instead of once per head.

RoPE is applied with a "swap projection" epilogue (see csr/snr), with head
dims host-permuted to [evens; odds] which leaves attention scores
invariant.
"""

import sys
import time
import numpy as np
import ml_dtypes
from contextlib import ExitStack

for _p in ("/opt/trn_rl_repo", "/root/.axon_site/_ro/trn_rl_repo"):
    if _p not in sys.path:
        sys.path.insert(0, _p)

import concourse.bass as bass
import concourse.tile as tile
from concourse import mybir

F32 = mybir.dt.float32
BF16 = mybir.dt.bfloat16
BF = ml_dtypes.bfloat16
AF = mybir.ActivationFunctionType
MUL = mybir.AluOpType.mult

B, S, D, H, DH = 2, 2048, 1024, 16, 64
FF = 2048
NCORES = 8
TLOC = 512                        # tokens per core in phase 3 (256 per batch)
THALF = 256
NQB = S // 512                    # 4 query blocks of 512 per batch
NDC = D // 128                    # 8 feature chunks
NFC = FF // 128                   # 16 ffn chunks
SCALE = 1.0 / float(np.sqrt(DH))
EPS = 1e-6
ROPE_BASE = 10000.0
SHROW = 130                       # rows per A2A shard: 2 heads x (64 num + 1 den)

_MAX_WAITS = 1


def _split_excess_waits(nc, max_waits=_MAX_WAITS):
    """walrus rejects >~2 sync-wait commands on one instruction; split the
    extras onto NoOps inserted just before, on the same engine."""
    counter = [0]

    def fresh_nop(engine, waits):
        counter[0] += 1
        nop = mybir.InstNoOp(name=f"I-waitsplit-{counter[0]}")
        nop.engine = engine
        nop.sync_info = mybir.SyncInfo(on_wait=list(waits), on_update=[])
        return nop

    for f in nc.m.functions:
        for bb in f.blocks:
            new_insts = []
            changed = False
            for inst in bb.instructions:
                si = inst.sync_info
                if si is not None and si.on_wait and len(si.on_wait) > max_waits:
                    waits = list(si.on_wait)
                    while len(waits) > max_waits:
                        chunk, waits = waits[:max_waits], waits[max_waits:]
                        new_insts.append(fresh_nop(inst.engine, chunk))
                    inst.sync_info = mybir.SyncInfo(
                        on_wait=waits, on_update=list(si.on_update or [])
                    )
                    changed = True
                new_insts.append(inst)
            if changed:
                bb.instructions[:] = new_insts


def _build_nc(debug=False):
    nc = bass.Bass("TRN2", target_bir_lowering=False, debug=False, num_devices=NCORES)

    xt_bf = nc.dram_tensor("xt_bf", [B * NQB, 128, NDC, 512], BF16, kind="ExternalInput")
    xt_loc = nc.dram_tensor("xt_loc", [2, 128, NDC, THALF], BF16, kind="ExternalInput")
    wqkv = nc.dram_tensor("wqkv", [128, 3, NDC, 128], BF16, kind="ExternalInput")
    wout = nc.dram_tensor("wout", [128, NDC, NDC, 128], BF16, kind="ExternalInput")
    wfc = nc.dram_tensor("wfc", [128, NDC, NFC, 128], BF16, kind="ExternalInput")
    wproj = nc.dram_tensor("wproj", [128, NFC, NDC, 128], BF16, kind="ExternalInput")
    cs_t = nc.dram_tensor("cs_t", [128, S], BF16, kind="ExternalInput")
    sn_t = nc.dram_tensor("sn_t", [128, S], BF16, kind="ExternalInput")
    wff = nc.dram_tensor("wff", [128, NDC], F32, kind="ExternalInput")
    ident_in = nc.dram_tensor("ident_in", [128, 128], BF16, kind="ExternalInput")
    sel16_in = nc.dram_tensor("sel16_in", [16, D], BF16, kind="ExternalInput")

    out_loc = nc.dram_tensor("out_loc", [2, 128, NDC, THALF], F32, kind="ExternalOutput")

    with tile.TileContext(nc) as tc, ExitStack() as top:
        # ---- single PSUM pool, tag-sized: 2x2(sc) + 2(num) + 2(misc) = 8 banks
        ps = top.enter_context(tc.tile_pool(name="ps", bufs=1, space="PSUM"))
        dram = top.enter_context(tc.tile_pool(name="dram", bufs=1, space="DRAM"))

        cc_in = [
            dram.tile([NCORES * SHROW, THALF], BF16, tag=f"cc_in{b}", name=f"cc_in{b}")
            for b in range(B)
        ]
        cc_out = [
            dram.tile([NCORES * SHROW, THALF], BF16, tag=f"cc_out{b}", name=f"cc_out{b}")
            for b in range(B)
        ]

        consts = top.enter_context(tc.tile_pool(name="consts", bufs=1))
        ones_col = consts.tile([128, 1], BF16, tag="ones_col")
        nc.vector.memset(ones_col[:], 1.0)
        ones_row = consts.tile([1, 128], BF16, tag="ones_row")
        nc.vector.memset(ones_row[:], 1.0)
        eps_sb = consts.tile([1, 1], F32, tag="eps_sb")
        nc.vector.memset(eps_sb[:], EPS)
        prime = consts.tile([1, 1], F32, tag="prime")
        nc.scalar.activation(out=prime[:], in_=eps_sb[:], func=AF.Ln)
        ident = consts.tile([128, 128], BF16, tag="ident")
        nc.gpsimd.dma_start(ident[:], ident_in[:])
        sel16 = consts.tile([16, NDC, 128], BF16, tag="sel16")
        nc.gpsimd.dma_start(sel16[:], sel16_in[:].rearrange("p (c m) -> p c m", m=128))

        wpre = top.enter_context(tc.tile_pool(name="wpre", bufs=1))
        wout_sb = wpre.tile([128, NDC, NDC, 128], BF16, tag="wout")
        wfc_sb = wpre.tile([128, NDC, NFC, 128], BF16, tag="wfc")
        wproj_sb = wpre.tile([128, NFC, NDC, 128], BF16, tag="wproj")

        # ================= phase 1: head-parallel attention =================
        persist = top.enter_context(tc.tile_pool(name="persist", bufs=1))
        cs_sb = persist.tile([128, S], BF16, tag="cs")
        sn_sb = persist.tile([128, S], BF16, tag="sn")
        wqkv_sb = persist.tile([128, 3, NDC, 128], BF16, tag="wqkv")
        nc.sync.dma_start(wqkv_sb[:], wqkv[:])

        bp = top.enter_context(tc.tile_pool(name="bp", bufs=1))
        xq = top.enter_context(tc.tile_pool(name="xq", bufs=3))
        rbc = top.enter_context(tc.tile_pool(name="rbc", bufs=3))
        tmp = top.enter_context(tc.tile_pool(name="tmp", bufs=2))
        sqp = top.enter_context(tc.tile_pool(name="sqp", bufs=1))
        tmp2 = top.enter_context(tc.tile_pool(name="tmp2", bufs=1))
        att_sb_pool = top.enter_context(tc.tile_pool(name="attsb", bufs=2))
        ptp = top.enter_context(tc.tile_pool(name="ptp", bufs=2))

        na_tiles = {}

        def norm_a(b, qb):
            """load x for query block (b, qb) and square it on the DVE (keeps
            the gpsimd queue free for the causal masks)."""
            qs = slice(qb * 512, (qb + 1) * 512)
            x_q = xq.tile([128, NDC, 512], BF16, tag="x_q", name="x_q")
            blk = b * NQB + qb
            nc.sync.dma_start(x_q[:, 0:4, :], xt_bf[blk, :, 0:4, :])
            nc.scalar.dma_start(x_q[:, 4:8, :], xt_bf[blk, :, 4:8, :])
            sq = sqp.tile([128, NDC, 512], BF16, tag="sq", name="sq")
            nc.vector.tensor_mul(sq[:], x_q[:], x_q[:])
            na_tiles[(b, qb)] = (x_q, sq)

        def norm_b(b, qb):
            """variance + rstd for a block whose x^2 is already in flight.
            rstd = exp(-0.5*ln(var/D + eps)): Ln+Exp share one table set."""
            x_q, sq = na_tiles.pop((b, qb))
            var_ps = ps.tile([1, 512], F32, tag="misc", name="var")
            for dc in range(NDC):
                nc.tensor.matmul(
                    var_ps[:], ones_col[:], sq[:, dc, :],
                    start=(dc == 0), stop=(dc == NDC - 1),
                )
            lnv = tmp.tile([1, 512], F32, tag="st", name="lnv")
            nc.scalar.activation(
                out=lnv[:], in_=var_ps[:], func=AF.Ln,
                scale=1.0 / D, bias=eps_sb[:],
            )
            rstd = tmp.tile([1, 512], BF16, tag="st", name="rstd")
            nc.scalar.activation(out=rstd[:], in_=lnv[:], func=AF.Exp, scale=-0.5)
            rstd_ps = ps.tile([128, 512], F32, tag="misc", name="rstd_ps")
            nc.tensor.matmul(rstd_ps[:], ones_row[:], rstd[:], start=True, stop=True)
            rstd_bc = rbc.tile([128, 512], BF16, tag="rstd_bc", name="rstd_bc")
            nc.vector.tensor_copy(rstd_bc[:], rstd_ps[:])
            h_tiles[(b, qb)] = (x_q, rstd_bc)

        steps = [(0, 0), (0, 1), (1, 0), (1, 1), (0, 2), (0, 3), (1, 2), (1, 3)]
        batch_tiles = {}
        h_tiles = {}

        def ensure_batch_tiles(b):
            if b not in batch_tiles:
                q_rot = bp.tile([128, S], BF16, tag="q_rot", name="q_rot", bufs=2)
                k_rot = bp.tile([128, S], BF16, tag="k_rot", name="k_rot", bufs=2)
                v_aug = bp.tile([128, S // 128, 2, 65], BF16, tag="v_aug", name="v_aug", bufs=2)
                nc.vector.memset(v_aug[:, :, :, 64:65], 1.0)
                batch_tiles[b] = (q_rot, k_rot, v_aug)
            return batch_tiles[b]

        SWAP16 = list(range(16, 32)) + list(range(16))

        def qkv_chunks(b, qb):
            """qkv projections + fused rmsnorm scaling + rope + v transpose,
            as a list of emission thunks so the work interleaves with the
            previous step's attention inner loop (fills exp-gated PE gaps)."""
            qs = slice(qb * 512, (qb + 1) * 512)
            q_rot, k_rot, v_aug = ensure_batch_tiles(b)
            st = {}

            def proj_half(rc, half, key):
                if half == 0:
                    st[key] = ps.tile([128, 512], F32, tag="proj", name=f"mm_{key}")
                for dc in range(half * 4, half * 4 + 4):
                    nc.tensor.matmul(
                        st[key][:], wqkv_sb[:, rc, dc, :], st["x_q"][:, dc, :],
                        start=(dc == 0), stop=(dc == NDC - 1),
                    )

            def c_start():
                st["x_q"], st["rstd_bc"] = h_tiles.pop((b, qb))
                rstd_bc = st["rstd_bc"]
                csr = tmp2.tile([128, 512], BF16, tag="csr", name="csr")
                nc.vector.tensor_mul(csr[:], cs_sb[:, qs], rstd_bc[:])
                snr = tmp2.tile([128, 512], BF16, tag="snr", name="snr")
                nc.vector.tensor_mul(snr[:], sn_sb[:, qs], rstd_bc[:])
                st["csr"], st["snr"] = csr, snr
                proj_half(0, 0, "q")

            def rope(key, dst):
                p_main = st[key]
                t1 = tmp2.tile([128, 512], BF16, tag="t1", name="t1", bufs=2)
                nc.vector.tensor_mul(t1[:], p_main[:], st["csr"][:])
                qsh = tmp2.tile([128, 512], F32, tag="qsh", name="qsh")
                nc.vector.stream_shuffle(qsh[:], p_main[:], SWAP16)
                t2 = tmp2.tile([128, 512], BF16, tag="t2", name="t2")
                nc.vector.tensor_mul(t2[:], qsh[:], st["snr"][:])
                nc.vector.tensor_add(dst[:, qs], t1[:], t2[:])

            def c_vepi():
                p_v = st["v"]
                v_f = tmp2.tile([128, 512], BF16, tag="t1", name="v_f", bufs=2)
                nc.vector.tensor_mul(v_f[:], p_v[:], st["rstd_bc"][:])
                for j in range(4):
                    kc = qb * 4 + j
                    tr_ps = ps.tile([128, 128], BF16, tag="misc", name="tr_ps")
                    nc.tensor.transpose(
                        tr_ps[:], v_f[:, j * 128:(j + 1) * 128], ident[:]
                    )
                    nc.vector.tensor_copy(
                        v_aug[:, kc, :, 0:64],
                        tr_ps[:].rearrange("p (h d) -> p h d", h=2),
                    )

            return [
                c_start,
                lambda: (proj_half(0, 1, "q"), rope("q", q_rot)),
                lambda: proj_half(1, 0, "k"),
                lambda: (proj_half(1, 1, "k"), rope("k", k_rot)),
                lambda: proj_half(2, 0, "v"),
                lambda: (proj_half(2, 1, "v"), c_vepi()),
            ]

        def attention_block(si, b, qb, feeds):
            qs = slice(qb * 512, (qb + 1) * 512)
            q_rot, k_rot, v_aug = batch_tiles[b]
            nkc = 4 * (qb + 1)
            num_h = [
                ps.tile([65, 512], F32, tag=f"num{hh}", name=f"num{hh}")
                for hh in range(2)
            ]
            for kc in range(nkc):
                ks = slice(kc * 128, (kc + 1) * 128)
                dlt = kc - qb * 4
                sc_ps = ps.tile([128, 2, 512], F32, tag="sc", name="sc_ps", bufs=2)
                for hh in range(2):
                    hs = slice(hh * 64, (hh + 1) * 64)
                    nc.tensor.matmul(
                        sc_ps[:, hh, :], k_rot[hs, ks], q_rot[hs, qs],
                        start=True, stop=True,
                        tile_position=(hh * 64, 0),
                    )
                pT = ptp.tile([128, 2, 512], BF16, tag="pT", name="pT")
                nc.scalar.activation(
                    out=pT[:], in_=sc_ps[:], func=AF.Exp, scale=SCALE
                )
                if dlt >= 0:
                    # causal: zero probs where query < key (same mask both heads)
                    nc.gpsimd.affine_select(
                        out=pT[:], in_=pT[:],
                        compare_op=mybir.AluOpType.is_ge,
                        fill=0.0, base=-dlt * 128,
                        channel_multiplier=-1,
                        pattern=[[0, 2], [1, 512]],
                    )
                # interleave next-step qkv/norm emission into the exp-gated gap
                n_feed = -(-len(feeds) // (nkc - kc))  # ceil
                for _ in range(n_feed):
                    feeds.pop(0)()
                for hh in range(2):
                    nc.tensor.matmul(
                        num_h[hh][:], v_aug[:, kc, hh, :], pT[:, hh, :],
                        start=(kc == 0), stop=(kc == nkc - 1),
                    )
            # ship un-normalised numerator + denominator row; divide in phase 3
            bi = 2 * b + (qb % 2)
            ccx = cc_in[0] if qb < 2 else cc_in[1]
            for hh in range(2):
                att65 = att_sb_pool.tile([65, 512], BF16, tag="att65", name="att65")
                nc.vector.tensor_copy(att65[:], num_h[hh][:])
                for half in range(2):
                    dst = 2 * bi + half
                    nc.sync.dma_start(
                        ccx[dst * SHROW + hh * 65: dst * SHROW + (hh + 1) * 65, :],
                        att65[:, half * THALF:(half + 1) * THALF],
                    )

        def emit_a2a(b):
            nc.gpsimd.collective_compute(
                "AllToAll",
                mybir.AluOpType.bypass,
                replica_groups=[list(range(NCORES))],
                ins=[cc_in[b][:]],
                outs=[cc_out[b][:]],
            )

        # ============ phase 3: token-parallel out-proj + FFN (one half) =====
        p3 = top.enter_context(tc.tile_pool(name="p3", bufs=1))
        p3t = top.enter_context(tc.tile_pool(name="p3t", bufs=2))
        wff_sb = p3.tile([128, NDC], F32, tag="wff")
        nc.scalar.dma_start(wff_sb[:], wff[:])

        p3_tiles = {}

        def phase3_load(b):
            cco = cc_out[b]
            at_all = p3.tile([128, NDC, THALF], BF16, tag="at_all")
            xl = p3.tile([128, NDC, THALF], BF16, tag="xl")
            for hh, eng in ((0, nc.sync), (1, nc.scalar)):
                eng.dma_start(
                    at_all[:].rearrange("(h r) c f -> h r c f", h=2)[hh],
                    cco[:].rearrange("(c h r) f -> h r c f", c=NCORES, h=2)[
                        hh, 0:64
                    ],
                )
            nc.gpsimd.dma_start(xl[:], xt_loc[b])
            # softmax denominators: 16 rows in one strided DMA. den_bf lives in
            # a phase-1 pool slot so the scheduler cannot hoist this DMA (and
            # the DVE chain behind it) ahead of the remaining phase-1 work.
            den_bf = p3.tile([16, THALF], BF16, tag="den_bf")
            nc.gpsimd.dma_start(
                den_bf[:],
                cco[:].rearrange("(c h r) f -> r (c h) f", c=NCORES, h=2)[64],
            )
            p3_tiles[b] = (at_all, xl, den_bf)

        def phase3_compute(b):
            at_all, xl, den_bf = p3_tiles.pop(b)
            den32 = p3.tile([16, THALF], F32, tag="den32")
            nc.vector.tensor_copy(den32[:], den_bf[:])
            rcp16 = p3.tile([16, THALF], F32, tag="rcp16")
            nc.vector.reciprocal(rcp16[:], den32[:])
            rcp16b = att_sb_pool.tile([16, THALF], BF16, tag="att65", name="rcp16b")
            nc.vector.tensor_copy(rcp16b[:], rcp16[:])
            # broadcast per-head reciprocal to 128 rows and normalise in place
            for dc in range(NDC):
                rcp_ps = ps.tile([128, THALF], F32, tag="num1", name="rcp_ps")
                nc.tensor.matmul(
                    rcp_ps[:], sel16[:, dc, :], rcp16b[:], start=True, stop=True
                )
                nc.vector.tensor_mul(at_all[:, dc, :], at_all[:, dc, :], rcp_ps[:])

            x1_all = p3.tile([128, NDC, THALF], BF16, tag="x1")
            h2_all = p3.tile([128, NDC, THALF], BF16, tag="h2", name="h2_all")
            hid_all = p3.tile([128, NFC, THALF], BF16, tag="hid")

            # out-proj + residual, with norm2 stats interleaved per chunk
            var2 = ps.tile([1, THALF], F32, tag="num0", name="var2")
            for ec in range(NDC):
                op_ps = ps.tile([128, 2, 512], F32, tag="sc", name="op_ps", bufs=2)
                for dc in range(NDC):
                    nc.tensor.matmul(
                        op_ps[:, 0, 0:THALF], wout_sb[:, dc, ec, :], at_all[:, dc, :],
                        start=(dc == 0), stop=(dc == NDC - 1),
                    )
                nc.vector.tensor_add(x1_all[:, ec, :], op_ps[:, 0, 0:THALF], xl[:, ec, :])
                sq2 = p3t.tile([128, THALF], BF16, tag="sq2", name="sq2")
                nc.vector.tensor_mul(sq2[:], x1_all[:, ec, :], x1_all[:, ec, :])
                nc.tensor.matmul(
                    var2[:], ones_col[:], sq2[:],
                    start=(ec == 0), stop=(ec == NDC - 1),
                )

            # rmsnorm 2 via ln/exp (same act table set as phase-1 exp)
            lnv2 = p3t.tile([1, THALF], F32, tag="st2", name="lnv2")
            nc.scalar.activation(
                out=lnv2[:], in_=var2[:], func=AF.Ln, scale=1.0 / D, bias=eps_sb[:]
            )
            rstd2 = p3t.tile([1, THALF], BF16, tag="st2", name="rstd2")
            nc.scalar.activation(out=rstd2[:], in_=lnv2[:], func=AF.Exp, scale=-0.5)
            rstd2_ps = ps.tile([128, THALF], F32, tag="misc", name="rstd2_ps")
            nc.tensor.matmul(rstd2_ps[:], ones_row[:], rstd2[:], start=True, stop=True)
            rstd2_bc = p3t.tile([128, THALF], BF16, tag="rstd2_bc")
            nc.vector.tensor_copy(rstd2_bc[:], rstd2_ps[:])
            for ec in range(NDC):
                nc.vector.scalar_tensor_tensor(
                    out=h2_all[:, ec, :],
                    in0=x1_all[:, ec, :],
                    scalar=wff_sb[:, ec:ec + 1],
                    in1=rstd2_bc[:],
                    op0=MUL, op1=MUL,
                )

            # fc + silu
            for fi in range(NFC):
                fc_ps = ps.tile([128, 2, 512], F32, tag="sc", name="fc_ps", bufs=2)
                for ec in range(NDC):
                    nc.tensor.matmul(
                        fc_ps[:, 0, 0:THALF], wfc_sb[:, ec, fi, :], h2_all[:, ec, :],
                        start=(ec == 0), stop=(ec == NDC - 1),
                    )
                nc.scalar.activation(
                    out=hid_all[:, fi, :], in_=fc_ps[:, 0, 0:THALF], func=AF.Silu
                )

            # proj + residual + store
            for ec in range(NDC):
                pr_ps = ps.tile([128, 2, 512], F32, tag="sc", name="pr_ps", bufs=2)
                for fi in range(NFC):
                    nc.tensor.matmul(
                        pr_ps[:, 0, 0:THALF], wproj_sb[:, fi, ec, :], hid_all[:, fi, :],
                        start=(fi == 0), stop=(fi == NFC - 1),
                    )
                y = p3t.tile([128, THALF], F32, tag="y")
                nc.vector.tensor_add(y[:], pr_ps[:, 0, 0:THALF], x1_all[:, ec, :])
                nc.sync.dma_start(out_loc[b, :, ec, :], y[:])

        # ================= emission schedule ===============================
        # prologue: x+x^2 for steps 0/1, stats for step 0, full qkv for step 0;
        # then attention(si) feeds [stats(si+1), x+x^2(si+2), qkv(si+1)] into
        # its exp-gated PE gaps chunk by chunk.
        norm_a(*steps[0])
        nc.gpsimd.dma_start(cs_sb[:], cs_t[:])
        nc.gpsimd.dma_start(sn_sb[:], sn_t[:])
        norm_b(*steps[0])
        for chunk in qkv_chunks(*steps[0]):
            chunk()
        norm_a(*steps[1])

        for si, (b, qb) in enumerate(steps):
            feeds = []
            if si + 1 < len(steps):
                sb, sqb = steps[si + 1]
                feeds.append(lambda sb=sb, sqb=sqb: norm_b(sb, sqb))
            if si + 2 < len(steps):
                sb, sqb = steps[si + 2]
                feeds.append(lambda sb=sb, sqb=sqb: norm_a(sb, sqb))
            if si + 1 < len(steps):
                feeds.extend(qkv_chunks(*steps[si + 1]))
            # big phase-3 weights: staggered so they don't starve the
            # startup-critical x/cs/sn loads of HBM bandwidth
            if si == 1:
                nc.gpsimd.dma_start(wout_sb[:], wout[:])
            if si == 2:
                nc.gpsimd.dma_start(wfc_sb[:], wfc[:])
            if si == 3:
                nc.gpsimd.dma_start(wproj_sb[:], wproj[:])
            attention_block(si, b, qb, feeds)
            if si == 3:
                emit_a2a(0)          # qb{0,1} redistribute, overlaps qb{2,3} attn

        with tc.tile_wait_until(5):
            phase3_load(0)
        emit_a2a(1)
        with tc.tile_wait_until(5):
            phase3_load(1)
            phase3_compute(0)        # overlaps the second AllToAll
            phase3_compute(1)

    _split_excess_waits(nc)
    return nc


# ---------------------------------------------------------------------------
# host-side prep


def _rope_tables():
    half = DH // 2
    inv_freq = 1.0 / (ROPE_BASE ** (2.0 * np.arange(half, dtype=np.float32) / DH))
    angles = np.arange(S, dtype=np.float32)[:, None] * inv_freq[None, :]  # (S, 32)
    cosT = np.cos(angles).T.astype(np.float32)  # (32, S) rows=freq
    sinT = np.sin(angles).T.astype(np.float32)
    # per head 64 rows = [e0..15, o0..15 | e16..31, o16..31]
    cs64 = np.concatenate([cosT[0:16], cosT[0:16], cosT[16:32], cosT[16:32]], axis=0)
    sn64 = np.concatenate([-sinT[0:16], sinT[0:16], -sinT[16:32], sinT[16:32]], axis=0)
    return (
        np.ascontiguousarray(np.tile(cs64, (2, 1))).astype(BF),
        np.ascontiguousarray(np.tile(sn64, (2, 1))).astype(BF),
    )  # (128, S)


def _prep_core_inputs(x, w_in_norm, w_ff_norm, w_qkv, w_out, w_fc, w_proj):
    x = np.asarray(x, dtype=np.float32)
    w_qkv = np.asarray(w_qkv, dtype=np.float32)
    w_out = np.asarray(w_out, dtype=np.float32)
    w_fc = np.asarray(w_fc, dtype=np.float32)
    w_proj = np.asarray(w_proj, dtype=np.float32)
    w_in_norm = np.asarray(w_in_norm, dtype=np.float32)
    w_ff_norm = np.asarray(w_ff_norm, dtype=np.float32)

    w_q, w_k, w_v = w_qkv[0:D], w_qkv[D:2 * D], w_qkv[2 * D:3 * D]

    xt = np.ascontiguousarray(x.transpose(0, 2, 1))        # (B, D, S)
    # block/partition-major: xt_pre[b*4+qb, p, c, f] = xt[b, c*128+p, qb*512+f],
    # so each 1MB x_q block DMA is contiguous per partition (8KB lines)
    xt_pre = np.ascontiguousarray(
        xt.reshape(B, NDC, 128, NQB, 512).transpose(0, 3, 2, 1, 4)
    ).reshape(B * NQB, 128, NDC, 512).astype(BF)

    cs_t, sn_t = _rope_tables()
    ident = np.eye(128, dtype=np.float32).astype(BF)
    # head selector for the phase-3 denominator broadcast: sel16[k, m] = [m//64 == k]
    sel16 = (np.arange(D)[None, :] // DH == np.arange(16)[:, None]).astype(BF)

    # SBUF layout [p, dc, ec, m]: element = w.T[dc*128+p, ec*128+m]
    wout_h = np.ascontiguousarray(
        w_out.T.reshape(NDC, 128, NDC, 128).transpose(1, 0, 2, 3)
    ).astype(BF)
    wfc_h = np.ascontiguousarray(
        w_fc.T.reshape(NDC, 128, NFC, 128).transpose(1, 0, 2, 3)
    ).astype(BF)
    wproj_h = np.ascontiguousarray(
        w_proj.T.reshape(NFC, 128, NDC, 128).transpose(1, 0, 2, 3)
    ).astype(BF)
    wff_h = np.ascontiguousarray(w_ff_norm.reshape(NDC, 128).T)

    ev = np.arange(0, DH, 2)
    od = np.arange(1, DH, 2)

    per_core = []
    for c in range(NCORES):
        hs = [2 * c, 2 * c + 1]

        def perm_rows(wm):
            # per head: [e0..15, o0..15, e16..31, o16..31]
            rows = []
            for h in hs:
                base = h * DH
                rows.append(wm[base + ev[0:16]])
                rows.append(wm[base + od[0:16]])
                rows.append(wm[base + ev[16:32]])
                rows.append(wm[base + od[16:32]])
            return np.concatenate(rows, axis=0)     # (128, D)

        def nat_rows(wm):
            return np.concatenate([wm[h * DH:(h + 1) * DH] for h in hs], axis=0)

        w_loc = np.stack([perm_rows(w_q), perm_rows(w_k), nat_rows(w_v)])  # (3, 128, D)
        w_loc = w_loc * w_in_norm[None, None, :]  # fold rmsnorm weight into qkv
        # SBUF layout [p, rc, dc, m]: element = w_loc[rc].T[dc*128+p, m]
        wqkv_h = np.ascontiguousarray(
            w_loc.transpose(0, 2, 1).reshape(3, NDC, 128, 128).transpose(2, 0, 1, 3)
        ).astype(BF)

        # phase-3 ownership: half x of core c = 256 tokens from query block
        # qb=(c//2)%2+2x of batch c//4, quarter (c%2); [x, p, c, f] layout
        b_c, qb_c, q0 = c // 4, (c // 2) % 2, (c % 2) * THALF
        xt_loc = np.stack(
            [
                xt[b_c, :, (qb_c + 2 * x) * 512 + q0: (qb_c + 2 * x) * 512 + q0 + THALF]
                .reshape(NDC, 128, THALF).transpose(1, 0, 2)
                for x in range(2)
            ]
        ).astype(BF)

        per_core.append({
            "xt_bf": xt_pre,
            "xt_loc": np.ascontiguousarray(xt_loc),
            "wqkv": wqkv_h,
            "wout": wout_h,
            "wfc": wfc_h,
            "wproj": wproj_h,
            "cs_t": cs_t,
            "sn_t": sn_t,
            "wff": wff_h,
            "ident_in": ident,
            "sel16_in": sel16,
        })
    return per_core


def _assemble(outs):
    full = np.empty((B, S, D), dtype=np.float32)
    for c in range(NCORES):
        b_c, qb_c, q0 = c // 4, (c // 2) % 2, (c % 2) * THALF
        for x in range(2):
            t0 = (qb_c + 2 * x) * 512 + q0
            # out_loc[x, p, cc, f] -> features cc*128+p
            full[b_c, t0:t0 + THALF, :] = (
                outs[c]["out_loc"][x].transpose(1, 0, 2).reshape(D, THALF).T
            )
    return full


_CACHE = {}


def _get_runner(debug=False):
    """Build the Bass module + a cached jitted shard_map executor, so repeated
    kernel() calls do not recompile."""
    key = ("runner", debug)
    if key in _CACHE:
        return _CACHE[key]

    nc = _build_nc(debug=debug)

    import jax
    from jax.sharding import Mesh, PartitionSpec
    from jax.experimental.shard_map import shard_map
    from concourse import bass2jax

    bass2jax.install_neuronx_cc_hook()

    in_names, out_names, out_avals, zero_outs = [], [], [], []
    for alloc in nc.m.functions[0].allocations:
        if not isinstance(alloc, mybir.MemoryLocationSet):
            continue
        name = alloc.memorylocations[0].name
        if alloc.kind == "ExternalInput":
            in_names.append(name)
        elif alloc.kind == "ExternalOutput":
            out_names.append(name)
            shape = tuple(alloc.tensor_shape)
            dtype = mybir.dt.np(alloc.dtype)
            out_avals.append(jax.core.ShapedArray(shape, dtype))
            zero_outs.append(np.zeros(shape, dtype))
    partition_name = nc.partition_id_tensor.name if nc.partition_id_tensor else None
    if partition_name is not None and partition_name in in_names:
        in_names.remove(partition_name)
    n_params = len(in_names)
    n_outs = len(out_avals)
    all_in_names = in_names + out_names
    if partition_name is not None:
        all_in_names = all_in_names + [partition_name]

    def _body(*args):
        operands = list(args)
        if partition_name is not None:
            operands.append(bass2jax.partition_id_tensor())
        outs = bass2jax._bass_exec_p.bind(
            *operands,
            out_avals=tuple(out_avals),
            in_names=tuple(all_in_names),
            out_names=tuple(out_names),
            lowering_input_output_aliases=(),
            sim_require_finite=True,
            sim_require_nnan=True,
            nc=nc,
        )
        return tuple(outs)

    devices = jax.devices()[:NCORES]
    mesh = Mesh(np.asarray(devices), ("core",))
    donate = tuple(range(n_params, n_params + n_outs))
    sharded = jax.jit(
        shard_map(
            _body,
            mesh=mesh,
            in_specs=(PartitionSpec("core"),) * (n_params + n_outs),
            out_specs=(PartitionSpec("core"),) * n_outs,
            check_rep=False,
        ),
        donate_argnums=donate,
        keep_unused=True,
    )

    def runner(in_maps):
        concat_in = [
            np.concatenate([np.asarray(m[name]) for m in in_maps], axis=0)
            for name in in_names
        ]
        concat_zeros = [
            np.zeros((NCORES * z.shape[0], *z.shape[1:]), z.dtype) for z in zero_outs
        ]
        out_arrs = sharded(*concat_in, *concat_zeros)
        return [
            {
                name: np.asarray(out_arrs[i]).reshape(NCORES, *out_avals[i].shape)[c]
                for i, name in enumerate(out_names)
            }
            for c in range(NCORES)
        ]

    _CACHE[key] = runner
    _CACHE[("runner_meta", debug)] = (sharded, in_names, out_avals, zero_outs, mesh)
    return runner


def kernel(**inputs) -> np.ndarray:
    per_core = _prep_core_inputs(**inputs)
    runner = _get_runner(debug=False)
    outs = runner(per_core)
    return _assemble(outs)


def time_kernel(iters=5, **inputs):
    """Wall-clock the jitted sharded execution with device-resident inputs.
    Returns best-of-iters nanoseconds (includes dispatch overhead, so it is
    an upper bound on HW kernel time)."""
    import jax

    per_core = _prep_core_inputs(**inputs)
    runner = _get_runner(debug=False)
    meta = _CACHE[("runner_meta", False)]
    sharded, in_names, out_avals, zero_outs, mesh = meta

    from jax.sharding import NamedSharding, PartitionSpec

    sh = NamedSharding(mesh, PartitionSpec("core"))
    concat_in = [
        np.concatenate([np.asarray(m[name]) for m in per_core], axis=0)
        for name in in_names
    ]
    dev_in = [jax.device_put(a, sh) for a in concat_in]

    def fresh_zeros():
        return [
            jax.device_put(
                np.zeros((NCORES * z.shape[0], *z.shape[1:]), z.dtype), sh
            )
            for z in zero_outs
        ]

    # warm
    out = sharded(*dev_in, *fresh_zeros())
    jax.block_until_ready(out)
    best = None
    for _ in range(iters):
        zs = fresh_zeros()
        jax.block_until_ready(zs)
        t0 = time.perf_counter_ns()
        out = sharded(*dev_in, *zs)
        jax.block_until_ready(out)
        t1 = time.perf_counter_ns()
        best = t1 - t0 if best is None else min(best, t1 - t0)
    return best


if __name__ == "__main__":
    rng = np.random.default_rng(0)
    ins = {
        "x": rng.standard_normal((B, S, D), dtype=np.float32),
        "w_in_norm": np.ones(D, np.float32),
        "w_ff_norm": np.ones(D, np.float32),
        "w_qkv": (rng.standard_normal((3 * D, D), dtype=np.float32) / 32),
        "w_out": (rng.standard_normal((D, D), dtype=np.float32) / 32),
        "w_fc": (rng.standard_normal((FF, D), dtype=np.float32) / 32),
        "w_proj": (rng.standard_normal((D, FF), dtype=np.float32) / np.sqrt(FF).astype(np.float32)),
    }
    out = kernel(**ins)
    print("out", out.shape, out.dtype, float(np.abs(out).mean()))
